# revision 32
# baseline (speedup 1.0000x reference)
"""MixFormer block kernel for 8 Trainium2 NeuronCores.

Sharding: data-parallel over batch B=16 -> 2 batch elements per core.
No collectives needed. Each core runs the full block (LN1 -> mixed
attention -> proj residual -> LN2 -> MLP residual) on its 2 batch
elements with bf16 matmuls and fp32 accumulation/residual path.

kernel(**inputs) takes the FULL inputs (as produced by the reference
setup_inputs) and returns the FULL [16, 980, 768] fp32 output.
"""

import os
import sys
import numpy as np

# ---------------------------------------------------------------- constants
B, N, C = 16, 980, 768
H, HD, HID = 12, 64, 3072
TLEN = 196  # t_h * t_w template tokens; search tokens attend to all N
EPS = 1e-5
NCORES = 8
PER = B // NCORES  # batch elements per core

NT = (N + 127) // 128  # 8 token tiles (7x128 + 84)
TOK_TILES = [(i * 128, min(128, N - i * 128)) for i in range(NT)]
TOK_CHUNKS = [(0, 490), (490, 490)]             # moving-dim chunks over tokens
V_CHUNKS = [(0, 512), (512, 256)]               # chunks over C=768 outputs
TMPL_KT = [(0, 128), (128, TLEN - 128)]         # key tiles for template region
TMPL_QCH = [(0, TLEN)]                          # template query chunk
SRCH_QCH = [(TLEN, 392), (TLEN + 392, 392)]     # search query chunks (784 = 2x392)


def _build_nc(hoist=True):
    import concourse.bass as bass
    import concourse.tile as tile
    import concourse.mybir as mybir
    from concourse.masks import make_identity
    from contextlib import ExitStack

    f32 = mybir.dt.float32
    bf16 = mybir.dt.bfloat16
    AF = mybir.ActivationFunctionType
    OP = mybir.AluOpType

    nc = bass.Bass()

    xs = nc.dram_tensor("xs", [PER, N, C], f32, kind="ExternalInput")
    wqk = nc.dram_tensor("wqk", [C, 2 * C], bf16, kind="ExternalInput")
    wv = nc.dram_tensor("wv", [C, C], bf16, kind="ExternalInput")
    wproj = nc.dram_tensor("wproj", [C, C], bf16, kind="ExternalInput")
    wfc1 = nc.dram_tensor("wfc1", [C, HID], bf16, kind="ExternalInput")
    wfc2 = nc.dram_tensor("wfc2", [HID, C], bf16, kind="ExternalInput")
    bqk = nc.dram_tensor("bqk", [2 * C], f32, kind="ExternalInput")
    bv = nc.dram_tensor("bv", [C], f32, kind="ExternalInput")
    bproj = nc.dram_tensor("bproj", [C], f32, kind="ExternalInput")
    bfc1 = nc.dram_tensor("bfc1", [HID], f32, kind="ExternalInput")
    bfc2 = nc.dram_tensor("bfc2", [C], f32, kind="ExternalInput")
    out_d = nc.dram_tensor("out", [PER, N, C], f32, kind="ExternalOutput")
    xmid_d = nc.dram_tensor("xmid", [PER, N, C], f32)  # internal scratch
    dnb_d = nc.dram_tensor("dnb", [48, 2, 512], f32)   # denom bounce buffer
    dnb2_d = nc.dram_tensor("dnb2", [48, 2, 512], f32)  # reciprocal bounce

    def layernorm_to_T(tc, tp, pst, src, tsz, t0, dstT, ident, eps_sb):
        """src: [tsz, 768] fp32 SBUF AP -> dstT[:, :, t0:t0+tsz] feature-major bf16."""
        stats = tp.tile([128, 3, 6], f32, tag="ln_st")
        for g in range(3):
            nc.vector.bn_stats(out=stats[:tsz, g], in_=src[:, g * 256:(g + 1) * 256])
        mv = tp.tile([128, 2], f32, tag="ln_mv")
        nc.vector.bn_aggr(out=mv[:tsz], in_=stats[:tsz])
        # rstd = exp(-0.5 * ln(var + eps)); keeps ACT in the ln/exp table set
        lnv = tp.tile([128, 1], f32, tag="ln_lnv")
        nc.scalar.activation(out=lnv[:tsz], in_=mv[:tsz, 1:2], func=AF.Ln,
                             bias=eps_sb[:tsz])
        rstd = tp.tile([128, 1], f32, tag="ln_rstd")
        nc.scalar.activation(out=rstd[:tsz], in_=lnv[:tsz], func=AF.Exp, scale=-0.5)
        ctr = tp.tile([128, C], f32, tag="ln_ctr")
        nc.vector.tensor_scalar_sub(ctr[:tsz], src, mv[:tsz, 0:1])
        lnt = tp.tile([128, C], bf16, tag="ln_out")
        nc.vector.tensor_scalar_mul(lnt[:tsz], ctr[:tsz], rstd[:tsz, 0:1])
        for c in range(6):
            pt = pst.tile([128, 128], bf16, tag="ln_tr")
            nc.tensor.transpose(pt[:, :tsz], lnt[:tsz, c * 128:(c + 1) * 128],
                                ident[:tsz, :tsz])
            nc.vector.tensor_copy(out=dstT[:, c, t0:t0 + tsz], in_=pt[:, :tsz])

    with tile.TileContext(nc) as tc, ExitStack() as top:
        persist = top.enter_context(tc.tile_pool(name="persist", bufs=1))
        ident = persist.tile([128, 128], bf16)
        make_identity(nc, ident)
        wqk_sb = persist.tile([128, 6, 2 * C], bf16)
        nc.sync.dma_start(out=wqk_sb, in_=wqk.rearrange("(k p) o -> p k o", p=128))
        wv_sb = persist.tile([128, 6, C], bf16)
        nc.sync.dma_start(out=wv_sb, in_=wv.rearrange("(k p) o -> p k o", p=128))
        wproj_sb = persist.tile([128, 6, C], bf16)
        nc.sync.dma_start(out=wproj_sb, in_=wproj.rearrange("(k p) o -> p k o", p=128))
        bqk_sb = persist.tile([128, 12], f32)
        nc.sync.dma_start(out=bqk_sb, in_=bqk.rearrange("(t p) -> p t", p=128))
        bfc1_sb = persist.tile([128, 24], f32)
        nc.sync.dma_start(out=bfc1_sb, in_=bfc1.rearrange("(t p) -> p t", p=128))
        bv_bc = persist.tile([128, C], f32)
        nc.sync.dma_start(out=bv_bc, in_=bv[:].partition_broadcast(128))
        bproj_bc = persist.tile([128, C], f32)
        nc.sync.dma_start(out=bproj_bc, in_=bproj[:].partition_broadcast(128))
        bfc2_bc = persist.tile([128, C], f32)
        nc.sync.dma_start(out=bfc2_bc, in_=bfc2[:].partition_broadcast(128))
        eps_sb = persist.tile([128, 1], f32)
        nc.vector.memset(eps_sb, EPS)

        for b in range(PER):
            with ExitStack() as bs:
                ln2p = bs.enter_context(tc.tile_pool(name=f"ln2p{b}", bufs=1))
                ln2T = ln2p.tile([128, 6, N], bf16)

                with ExitStack() as asx:
                    abuf = asx.enter_context(tc.tile_pool(name=f"abuf{b}", bufs=1))
                    x_sb = abuf.tile([128, NT, C], f32)
                    ln1T = abuf.tile([128, 6, N], bf16)
                    qkT = abuf.tile([128, 12, N], bf16)   # q o-tiles 0..5, k 6..11
                    v_sb = abuf.tile([128, NT, H, 65], bf16)  # col 64 = ones
                    xatt = abuf.tile([128, 6, N], bf16)   # feature-major attn out

                    # ---- A1: load x, LN1, transpose to feature-major ----
                    with ExitStack() as ph:
                        tp = ph.enter_context(tc.tile_pool(name=f"a1t{b}", bufs=3))
                        pst = ph.enter_context(
                            tc.tile_pool(name=f"a1p{b}", bufs=3, space="PSUM"))
                        for t, (t0, tsz) in enumerate(TOK_TILES):
                            nc.sync.dma_start(out=x_sb[:tsz, t], in_=xs[b, t0:t0 + tsz])
                            layernorm_to_T(tc, tp, pst, x_sb[:tsz, t], tsz, t0,
                                           ln1T, ident, eps_sb)

                    # ---- A2: qkv projections ----
                    with ExitStack() as ph:
                        psqk = ph.enter_context(
                            tc.tile_pool(name=f"a2q{b}", bufs=4, space="PSUM"))
                        psv = ph.enter_context(
                            tc.tile_pool(name=f"a2v{b}", bufs=4, space="PSUM"))
                        # q^T, k^T feature-major [o, tok]
                        for ot in range(12):
                            qps = [psqk.tile([128, 512], f32, tag="qk",
                                             name=f"qk{ot}_{ci}")
                                   for ci in range(2)]
                            for k in range(6):
                                for ci, (c0, csz) in enumerate(TOK_CHUNKS):
                                    nc.tensor.matmul(
                                        qps[ci][:, :csz],
                                        wqk_sb[:, k, ot * 128:(ot + 1) * 128],
                                        ln1T[:, k, c0:c0 + csz],
                                        start=(k == 0), stop=(k == 5))
                            for ci, (c0, csz) in enumerate(TOK_CHUNKS):
                                nc.vector.tensor_scalar_add(
                                    qkT[:, ot, c0:c0 + csz], qps[ci][:, :csz],
                                    bqk_sb[:, ot:ot + 1])
                        # v token-major with per-head stride-65 layout + ones col
                        nc.vector.memset(v_sb[:, :, :, 64:65], 1.0)
                        for t, (t0, tsz) in enumerate(TOK_TILES):
                            vps = [psv.tile([128, 512], f32, tag="v",
                                            name=f"v{t}_{ci}")
                                   for ci in range(2)]
                            for k in range(6):
                                for ci, (c0, csz) in enumerate(V_CHUNKS):
                                    nc.tensor.matmul(
                                        vps[ci][:tsz, :csz],
                                        ln1T[:, k, t0:t0 + tsz],
                                        wv_sb[:, k, c0:c0 + csz],
                                        start=(k == 0), stop=(k == 5))
                            for ci, (c0, csz) in enumerate(V_CHUNKS):
                                nc.vector.tensor_add(
                                    v_sb[:tsz, t, c0 // 64:(c0 + csz) // 64, 0:64],
                                    vps[ci][:tsz, :csz].rearrange(
                                        "p (h d) -> p h d", d=64),
                                    bv_bc[:tsz, c0:c0 + csz].rearrange(
                                        "p (h d) -> p h d", d=64))

                    # ---- A3: attention per head pair ----
                    with ExitStack() as ph:
                        pss = ph.enter_context(
                            tc.tile_pool(name=f"a3s{b}", bufs=1, space="PSUM"))
                        psa = ph.enter_context(
                            tc.tile_pool(name=f"a3a{b}", bufs=1, space="PSUM"))
                        ptp = ph.enter_context(tc.tile_pool(name=f"a3p{b}", bufs=6))
                        dnp = ph.enter_context(tc.tile_pool(name=f"a3d{b}", bufs=3))
                        for hp in range(6):
                            for (kts, qchunks) in ((TMPL_KT, TMPL_QCH),
                                                   (TOK_TILES, SRCH_QCH)):
                                nch = len(qchunks)
                                av = [psa.tile([128, 2, 512], f32, tag=f"av{s}",
                                                name=f"av{s}")
                                      for s in (0, 1)]
                                nkt = len(kts)
                                for ki, (k0, ksz) in enumerate(kts):
                                    kt_t, ko = k0 // 128, k0 % 128
                                    qsz0 = qchunks[0][1]
                                    for s in (0, 1):
                                        pb = s * 64
                                        h = 2 * hp + s
                                        pt = ptp.tile([128, nch, 512], bf16,
                                                      tag=f"pt{s}")
                                        ss = pss.tile([128, 2, 512], f32,
                                                      tag=f"s{s}")
                                        for ci, (q0, qsz) in enumerate(qchunks):
                                            # S^T[kt,qt] = k_h^T.T @ q_h^T (K=64,
                                            # row group s) — pairs run concurrently
                                            nc.tensor.matmul(
                                                ss[:ksz, ci, :qsz],
                                                qkT[pb:pb + 64, 6 + hp, k0:k0 + ksz],
                                                qkT[pb:pb + 64, hp, q0:q0 + qsz],
                                                start=True, stop=True)
                                        # one exp over all chunks of this key tile
                                        nc.scalar.activation(
                                            pt[:ksz, :nch, :qsz0],
                                            ss[:ksz, :nch, :qsz0], AF.Exp)
                                        for ci, (q0, qsz) in enumerate(qchunks):
                                            # unnormalized AV; lhsT col 64 is ones
                                            # -> row 64 of psum = softmax denom
                                            nc.tensor.matmul(
                                                av[s][0:65, ci, :qsz],
                                                v_sb[ko:ko + ksz, kt_t, h, 0:65],
                                                pt[:ksz, ci, :qsz],
                                                start=(ki == 0), stop=(ki == nkt - 1))
                                # normalize: xatt_h = av[0:64] * (1/av[64])
                                for s in (0, 1):
                                    qsz0 = qchunks[0][1]
                                    # copy psum -> SBUF promptly so the next
                                    # head pair's AV matmuls get the banks
                                    avs = dnp.tile([65, 2, 512], f32,
                                                   tag=f"avs{s}")
                                    nc.vector.tensor_copy(
                                        avs[:, :nch, :qsz0],
                                        av[s][0:65, :nch, :qsz0])
                                    bc = dnp.tile([64, 2, 512], f32, tag="bc")
                                    di = ((b * 6 + hp) * 2 + (nch - 1)) * 2 + s
                                    # denom row -> DRAM, reload spread over 49
                                    # partitions so the 8-pass reciprocal runs
                                    # on a short free dim, then bounce back and
                                    # broadcast across 64 partitions (DMA-only)
                                    fr = qsz0 // 49
                                    nc.sync.dma_start(
                                        out=dnb_d[di:di + 1, :nch, :qsz0],
                                        in_=avs[64:65, :nch, :qsz0])
                                    dn_t = dnp.tile([49, 2, 16], f32, tag="dnt")
                                    nc.sync.dma_start(
                                        out=dn_t[:, :nch, :fr],
                                        in_=dnb_d[di, :nch, :qsz0].rearrange(
                                            "a (p f) -> p a f", p=49))
                                    dn_r = dnp.tile([49, 2, 16], f32, tag="dnr")
                                    nc.vector.reciprocal(dn_r[:, :nch, :fr],
                                                         dn_t[:, :nch, :fr])
                                    nc.sync.dma_start(
                                        out=dnb2_d[di, :nch, :qsz0].rearrange(
                                            "a (p f) -> p a f", p=49),
                                        in_=dn_r[:, :nch, :fr])
                                    nc.sync.dma_start(
                                        out=bc[:, :nch, :qsz0],
                                        in_=dnb2_d[di, :nch, :qsz0]
                                        .partition_broadcast(64))
                                    if s == 0:
                                        for ci, (q0, qsz) in enumerate(qchunks):
                                            nc.vector.tensor_mul(
                                                xatt[0:64, hp, q0:q0 + qsz],
                                                avs[0:64, ci, :qsz],
                                                bc[:, ci, :qsz])
                                    else:
                                        stg = dnp.tile([64, 2, 512], bf16, tag="stg")
                                        for ci, (q0, qsz) in enumerate(qchunks):
                                            nc.vector.tensor_mul(
                                                stg[:, ci, :qsz],
                                                avs[0:64, ci, :qsz],
                                                bc[:, ci, :qsz])
                                        for ci, (q0, qsz) in enumerate(qchunks):
                                            # odd head rows live at partitions 64+
                                            nc.sync.dma_start(
                                                out=xatt[64:128, hp, q0:q0 + qsz],
                                                in_=stg[:, ci, :qsz])

                    # ---- A4+A5: proj + residual -> xmid; LN2 ----
                    with ExitStack() as ph:
                        psp = ph.enter_context(
                            tc.tile_pool(name=f"a4p{b}", bufs=4, space="PSUM"))
                        pst = ph.enter_context(
                            tc.tile_pool(name=f"a4t{b}", bufs=3, space="PSUM"))
                        tp = ph.enter_context(tc.tile_pool(name=f"a4s{b}", bufs=3))
                        for t, (t0, tsz) in enumerate(TOK_TILES):
                            xm = tp.tile([128, C], f32, tag="xm")
                            pps = [psp.tile([128, 512], f32, tag="pj",
                                            name=f"pj{t}_{ci}")
                                   for ci in range(2)]
                            for k in range(6):
                                for ci, (c0, csz) in enumerate(V_CHUNKS):
                                    nc.tensor.matmul(
                                        pps[ci][:tsz, :csz],
                                        xatt[:, k, t0:t0 + tsz],
                                        wproj_sb[:, k, c0:c0 + csz],
                                        start=(k == 0), stop=(k == 5))
                            for ci, (c0, csz) in enumerate(V_CHUNKS):
                                nc.vector.tensor_add(
                                    xm[:tsz, c0:c0 + csz], pps[ci][:tsz, :csz],
                                    x_sb[:tsz, t, c0:c0 + csz])
                            nc.vector.tensor_add(xm[:tsz], xm[:tsz], bproj_bc[:tsz])
                            nc.sync.dma_start(out=xmid_d[b, t0:t0 + tsz], in_=xm[:tsz])
                            layernorm_to_T(tc, tp, pst, xm[:tsz], tsz, t0,
                                           ln2T, ident, eps_sb)

                # ---- B: MLP (attention buffers released) ----
                with ExitStack() as ph:
                    mw = ph.enter_context(tc.tile_pool(name=f"mw{b}", bufs=1))
                    wf1 = mw.tile([128, 6, HID], bf16)
                    nc.sync.dma_start(out=wf1,
                                      in_=wfc1.rearrange("(k p) o -> p k o", p=128))
                    wf2 = mw.tile([128, 24, C], bf16)
                    nc.sync.dma_start(out=wf2,
                                      in_=wfc2.rearrange("(k p) o -> p k o", p=128))
                    hT = mw.tile([128, 24, N], bf16)
                    ps1 = ph.enter_context(
                        tc.tile_pool(name=f"b1p{b}", bufs=2, space="PSUM"))
                    ps2 = ph.enter_context(
                        tc.tile_pool(name=f"b2p{b}", bufs=4, space="PSUM"))
                    tpm = ph.enter_context(tc.tile_pool(name=f"bt{b}", bufs=3))
                    for ot in range(24):
                        ps = ps1.tile([128, 2, 512], f32, tag="f1")
                        for k in range(6):
                            for ci, (c0, csz) in enumerate(TOK_CHUNKS):
                                nc.tensor.matmul(
                                    ps[:, ci, :csz],
                                    wf1[:, k, ot * 128:(ot + 1) * 128],
                                    ln2T[:, k, c0:c0 + csz],
                                    start=(k == 0), stop=(k == 5))
                        nc.scalar.activation(
                            hT[:, ot, :].rearrange("p (a c) -> p a c", c=490),
                            ps[:, :, :490], AF.Gelu,
                            bias=bfc1_sb[:, ot:ot + 1])
                    for t, (t0, tsz) in enumerate(TOK_TILES):
                        xm = tpm.tile([128, C], f32, tag="xm2")
                        nc.sync.dma_start(out=xm[:tsz], in_=xmid_d[b, t0:t0 + tsz])
                        ot_t = tpm.tile([128, C], f32, tag="ott")
                        fps = [ps2.tile([128, 512], f32, tag="f2",
                                        name=f"f2{t}_{ci}")
                               for ci in range(2)]
                        for k in range(24):
                            for ci, (c0, csz) in enumerate(V_CHUNKS):
                                nc.tensor.matmul(
                                    fps[ci][:tsz, :csz],
                                    hT[:, k, t0:t0 + tsz],
                                    wf2[:, k, c0:c0 + csz],
                                    start=(k == 0), stop=(k == 23))
                        for ci, (c0, csz) in enumerate(V_CHUNKS):
                            nc.vector.tensor_add(
                                ot_t[:tsz, c0:c0 + csz], fps[ci][:tsz, :csz],
                                xm[:tsz, c0:c0 + csz])
                        nc.vector.tensor_add(ot_t[:tsz], ot_t[:tsz], bfc2_bc[:tsz])
                        nc.sync.dma_start(out=out_d[b, t0:t0 + tsz], in_=ot_t[:tsz])

    if hoist:
        _hoist_excess_waits(nc, mybir)
    return nc


def _hoist_excess_waits(nc, mybir, cap=1, nop_cap=1):
    """walrus's 64B instruction encodings fit only ~1 sync-wait command for
    operand-heavy structs (TS/AC/...). Move excess waits onto same-engine
    NoOps inserted right before the instruction."""
    skip = ("InstNoOp", "InstEventSemaphore", "InstCall",
            "InstAllEngineBarrier", "InstUnconditionalBranch", "InstISA")
    n = 0
    for f in nc.m.functions:
        for blk in getattr(f, "blocks", []):
            out = []
            for inst in blk.instructions:
                si = inst.sync_info
                if (si is not None and len(si.on_wait) > cap
                        and type(inst).__name__ not in skip):
                    waits = list(si.on_wait)
                    keep, extra = waits[:cap], waits[cap:]
                    while extra:
                        chunk, extra = extra[:nop_cap], extra[nop_cap:]
                        n += 1
                        out.append(mybir.InstNoOp(
                            name=f"nopw-{n}", engine=inst.engine, ins=[], outs=[],
                            sync_info=mybir.SyncInfo(on_wait=chunk, on_update=[])))
                    inst.sync_info = mybir.SyncInfo(
                        on_wait=keep, on_update=list(si.on_update))
                out.append(inst)
            blk.instructions = out


def _prep_inputs(inputs):
    """Host-side weight folding; returns dict of per-core-constant arrays."""
    import ml_dtypes
    f32 = np.float32
    ln1_w = np.asarray(inputs["ln1_w"], f32)
    ln1_b = np.asarray(inputs["ln1_b"], f32)
    ln2_w = np.asarray(inputs["ln2_w"], f32)
    ln2_b = np.asarray(inputs["ln2_b"], f32)
    qkv_w = np.asarray(inputs["qkv_w"], f32)
    qkv_b = np.asarray(inputs["qkv_b"], f32)
    proj_w = np.asarray(inputs["proj_w"], f32)
    proj_b = np.asarray(inputs["proj_b"], f32)
    fc1_w = np.asarray(inputs["fc1_w"], f32)
    fc1_b = np.asarray(inputs["fc1_b"], f32)
    fc2_w = np.asarray(inputs["fc2_w"], f32)
    fc2_b = np.asarray(inputs["fc2_b"], f32)

    scale = HD ** -0.5
    # fold LN1 affine into qkv; fold attention scale into q
    w_full = ln1_w[:, None] * qkv_w            # [C, 3C]
    b_full = qkv_b + ln1_b @ qkv_w             # [3C]
    w_full = w_full.copy()
    b_full = b_full.copy()
    w_full[:, :C] *= scale
    b_full[:C] *= scale
    wqk = w_full[:, :2 * C]
    bqk = b_full[:2 * C]
    wv = w_full[:, 2 * C:]
    bv = b_full[2 * C:]
    # fold LN2 affine into fc1
    wfc1 = ln2_w[:, None] * fc1_w
    bfc1 = fc1_b + ln2_b @ fc1_w

    bf16 = ml_dtypes.bfloat16
    return {
        "wqk": np.ascontiguousarray(wqk, dtype=bf16),
        "wv": np.ascontiguousarray(wv, dtype=bf16),
        "wproj": np.ascontiguousarray(proj_w, dtype=bf16),
        "wfc1": np.ascontiguousarray(wfc1, dtype=bf16),
        "wfc2": np.ascontiguousarray(fc2_w, dtype=bf16),
        "bqk": np.ascontiguousarray(bqk, dtype=f32),
        "bv": np.ascontiguousarray(bv, dtype=f32),
        "bproj": np.ascontiguousarray(proj_b, dtype=f32),
        "bfc1": np.ascontiguousarray(bfc1, dtype=f32),
        "bfc2": np.ascontiguousarray(fc2_b, dtype=f32),
    }


def _enable_axon_trace():
    """Register the NTFF profile hook that this image's antenv lacks."""
    import types
    from trn_agent_boot.trn_boot import _ntff_profile_via_ctypes
    mod = types.ModuleType("antenv.axon_hooks")
    hook = _ntff_profile_via_ctypes("/opt/axon/libaxon_pjrt.so")
    mod.get_axon_ntff_profile_hook = lambda: hook
    mod.set_axon_ntff_profile_hook = lambda h: None
    sys.modules["antenv.axon_hooks"] = mod
    import concourse.bass_utils as bu
    bu.upload_artifacts = lambda tmpdir: tmpdir  # no artifact bucket here


def _run_on_device(x_full, consts, trace=False):
    """Build + run the SPMD kernel on the 8 cores. Returns (out, exec_ns)."""
    sys.path.insert(0, "/opt/trn_rl_repo")
    from concourse.bass_utils import run_bass_kernel_spmd

    if os.environ.get("BASS_LDW_OPT", "0") == "1":
        import concourse.bass_utils as bu
        if not getattr(bu, "_ldw_patched", False):
            orig_run = bu.run_command

            def _run_ldw(argv, **kw):
                argv = ["--enable-ldw-opt=true"
                        if a == "--enable-ldw-opt=false" else a for a in argv]
                return orig_run(argv, **kw)

            bu.run_command = _run_ldw
            bu._ldw_patched = True

    tmpdir = None
    if trace:
        try:
            _enable_axon_trace()
            tmpdir = os.environ.get("BASS_KERNEL_TRACE_DIR")
            if tmpdir:
                import shutil
                shutil.rmtree(tmpdir, ignore_errors=True)
                os.makedirs(tmpdir, exist_ok=True)
        except Exception as e:  # profiling is best-effort
            print("trace hook setup failed:", e, file=sys.stderr)
            trace = False

    nc = _build_nc()
    in_maps = []
    for core in range(NCORES):
        m = dict(consts)
        m["xs"] = np.ascontiguousarray(
            x_full[core * PER:(core + 1) * PER], dtype=np.float32)
        in_maps.append(m)
    try:
        res = run_bass_kernel_spmd(nc, in_maps, list(range(NCORES)),
                                   trace=trace, tmpdir=tmpdir)
    except Exception:
        if not trace:
            raise
        print("traced run failed; retrying without trace", file=sys.stderr)
        res = run_bass_kernel_spmd(nc, in_maps, list(range(NCORES)), trace=False)
    out = np.concatenate([r["out"] for r in res.results], axis=0)
    return out, res.exec_time_ns


def _subproc_main(tmpdir):
    import ml_dtypes
    data = np.load(os.path.join(tmpdir, "in.npz"))
    consts = {}
    for k in data.files:
        if k == "x":
            continue
        if k.endswith("__bf16"):
            consts[k[:-6]] = data[k].view(ml_dtypes.bfloat16)
        else:
            consts[k] = data[k]
    trace = os.environ.get("BASS_KERNEL_TRACE", "0") == "1"
    out, exec_ns = _run_on_device(data["x"], consts, trace=trace)
    np.savez(os.path.join(tmpdir, "out.npz"), out=out,
             exec_ns=np.int64(exec_ns if exec_ns else -1))


LAST_EXEC_NS = None


def kernel(**inputs):
    global LAST_EXEC_NS
    x = np.asarray(inputs["x"], np.float32)
    assert x.shape == (B, N, C), x.shape
    t_h = int(np.asarray(inputs.get("t_h", 14)))
    t_w = int(np.asarray(inputs.get("t_w", 14)))
    assert t_h * t_w == TLEN, (t_h, t_w)
    consts = _prep_inputs(inputs)

    # Run the device part in a subprocess with a clean JAX platform env, so a
    # harness that pinned JAX_PLATFORMS=cpu (for the reference) doesn't break
    # the PJRT/axon execution path.
    import subprocess
    import tempfile
    with tempfile.TemporaryDirectory() as td:
        saved = {}
        for k, v in consts.items():
            if v.dtype == np.float32:
                saved[k] = v
            else:  # bfloat16 -> ship as uint16 bits
                saved[k + "__bf16"] = v.view(np.uint16)
        np.savez(os.path.join(td, "in.npz"), x=x, **saved)
        env = dict(os.environ)
        env.pop("JAX_PLATFORMS", None)
        pyp = env.get("PYTHONPATH", "")
        here = os.path.dirname(os.path.abspath(__file__))
        env["PYTHONPATH"] = ":".join(p for p in [here, "/opt/trn_rl_repo", pyp] if p)
        subprocess.run(
            [sys.executable, "-c",
             f"import kernel; kernel._subproc_main({td!r})"],
            check=True, env=env)
        data = np.load(os.path.join(td, "out.npz"))
        out = data["out"]
        LAST_EXEC_NS = int(data["exec_ns"])
    return out.astype(np.float32)


if __name__ == "__main__":
    if len(sys.argv) > 1 and sys.argv[1] == "_sub":
        _subproc_main(sys.argv[2])


# revision 33
# speedup vs baseline: 1.1664x; 1.1664x over previous
"""MixFormer block kernel for 8 Trainium2 NeuronCores.

Sharding: data-parallel over batch B=16 -> 2 batch elements per core.
No collectives needed. Each core runs the full block (LN1 -> mixed
attention -> proj residual -> LN2 -> MLP residual) on its 2 batch
elements with bf16 matmuls and fp32 accumulation/residual path.

kernel(**inputs) takes the FULL inputs (as produced by the reference
setup_inputs) and returns the FULL [16, 980, 768] fp32 output.
"""

import os
import sys
import numpy as np

# ---------------------------------------------------------------- constants
B, N, C = 16, 980, 768
H, HD, HID = 12, 64, 3072
TLEN = 196  # t_h * t_w template tokens; search tokens attend to all N
EPS = 1e-5
NCORES = 8
PER = B // NCORES  # batch elements per core

NT = (N + 127) // 128  # 8 token tiles (7x128 + 84)
TOK_TILES = [(i * 128, min(128, N - i * 128)) for i in range(NT)]
TOK_CHUNKS = [(0, 490), (490, 490)]             # moving-dim chunks over tokens
V_CHUNKS = [(0, 512), (512, 256)]               # chunks over C=768 outputs
TMPL_KT = [(0, 128), (128, TLEN - 128)]         # key tiles for template region
TMPL_QCH = [(0, TLEN)]                          # template query chunk
SRCH_QCH = [(TLEN, 392), (TLEN + 392, 392)]     # search query chunks (784 = 2x392)


def _build_nc(hoist=True):
    import concourse.bass as bass
    import concourse.tile as tile
    import concourse.mybir as mybir
    from concourse.masks import make_identity
    from contextlib import ExitStack

    f32 = mybir.dt.float32
    bf16 = mybir.dt.bfloat16
    AF = mybir.ActivationFunctionType
    OP = mybir.AluOpType

    nc = bass.Bass()

    xs = nc.dram_tensor("xs", [PER, N, C], f32, kind="ExternalInput")
    wqk = nc.dram_tensor("wqk", [C, 2 * C], bf16, kind="ExternalInput")
    wv = nc.dram_tensor("wv", [C, C], bf16, kind="ExternalInput")
    wproj = nc.dram_tensor("wproj", [C, C], bf16, kind="ExternalInput")
    wfc1 = nc.dram_tensor("wfc1", [C, HID], bf16, kind="ExternalInput")
    wfc2 = nc.dram_tensor("wfc2", [HID, C], bf16, kind="ExternalInput")
    bqk = nc.dram_tensor("bqk", [2 * C], f32, kind="ExternalInput")
    bv = nc.dram_tensor("bv", [C], f32, kind="ExternalInput")
    bproj = nc.dram_tensor("bproj", [C], f32, kind="ExternalInput")
    bfc1 = nc.dram_tensor("bfc1", [HID], f32, kind="ExternalInput")
    bfc2 = nc.dram_tensor("bfc2", [C], f32, kind="ExternalInput")
    out_d = nc.dram_tensor("out", [PER, N, C], f32, kind="ExternalOutput")
    xmid_d = nc.dram_tensor("xmid", [PER, N, C], f32)  # internal scratch
    dnb_d = nc.dram_tensor("dnb", [48, 2, 512], f32)   # denom bounce buffer
    dnb2_d = nc.dram_tensor("dnb2", [48, 2, 512], f32)  # reciprocal bounce

    def layernorm_to_T(tc, tp, pst, src, tsz, t0, dstT, ident, eps_sb):
        """src: [tsz, 768] fp32 SBUF AP -> dstT[:, :, t0:t0+tsz] feature-major bf16."""
        stats = tp.tile([128, 3, 6], f32, tag="ln_st")
        for g in range(3):
            nc.vector.bn_stats(out=stats[:tsz, g], in_=src[:, g * 256:(g + 1) * 256])
        mv = tp.tile([128, 2], f32, tag="ln_mv")
        nc.vector.bn_aggr(out=mv[:tsz], in_=stats[:tsz])
        # rstd = exp(-0.5 * ln(var + eps)); keeps ACT in the ln/exp table set
        lnv = tp.tile([128, 1], f32, tag="ln_lnv")
        nc.scalar.activation(out=lnv[:tsz], in_=mv[:tsz, 1:2], func=AF.Ln,
                             bias=eps_sb[:tsz])
        rstd = tp.tile([128, 1], f32, tag="ln_rstd")
        nc.scalar.activation(out=rstd[:tsz], in_=lnv[:tsz], func=AF.Exp, scale=-0.5)
        ctr = tp.tile([128, C], f32, tag="ln_ctr")
        nc.vector.tensor_scalar_sub(ctr[:tsz], src, mv[:tsz, 0:1])
        lnt = tp.tile([128, C], bf16, tag="ln_out")
        nc.vector.tensor_scalar_mul(lnt[:tsz], ctr[:tsz], rstd[:tsz, 0:1])
        for c in range(6):
            pt = pst.tile([128, 128], bf16, tag="ln_tr")
            nc.tensor.transpose(pt[:, :tsz], lnt[:tsz, c * 128:(c + 1) * 128],
                                ident[:tsz, :tsz])
            nc.vector.tensor_copy(out=dstT[:, c, t0:t0 + tsz], in_=pt[:, :tsz])

    with tile.TileContext(nc) as tc, ExitStack() as top:
        persist = top.enter_context(tc.tile_pool(name="persist", bufs=1))
        ident = persist.tile([128, 128], bf16)
        make_identity(nc, ident)
        wqk_sb = persist.tile([128, 6, 2 * C], bf16)
        nc.sync.dma_start(out=wqk_sb, in_=wqk.rearrange("(k p) o -> p k o", p=128))
        wv_sb = persist.tile([128, 6, C], bf16)
        nc.sync.dma_start(out=wv_sb, in_=wv.rearrange("(k p) o -> p k o", p=128))
        wproj_sb = persist.tile([128, 6, C], bf16)
        nc.sync.dma_start(out=wproj_sb, in_=wproj.rearrange("(k p) o -> p k o", p=128))
        bqk_sb = persist.tile([128, 12], f32)
        nc.sync.dma_start(out=bqk_sb, in_=bqk.rearrange("(t p) -> p t", p=128))
        bfc1_sb = persist.tile([128, 24], f32)
        nc.sync.dma_start(out=bfc1_sb, in_=bfc1.rearrange("(t p) -> p t", p=128))
        bv_bc = persist.tile([128, C], f32)
        nc.sync.dma_start(out=bv_bc, in_=bv[:].partition_broadcast(128))
        bproj_bc = persist.tile([128, C], f32)
        nc.sync.dma_start(out=bproj_bc, in_=bproj[:].partition_broadcast(128))
        bfc2_bc = persist.tile([128, C], f32)
        nc.sync.dma_start(out=bfc2_bc, in_=bfc2[:].partition_broadcast(128))
        eps_sb = persist.tile([128, 1], f32)
        nc.vector.memset(eps_sb, EPS)

        for b in range(PER):
            with ExitStack() as bs:
                ln2p = bs.enter_context(tc.tile_pool(name=f"ln2p{b}", bufs=1))
                ln2T = ln2p.tile([128, 6, N], bf16)

                with ExitStack() as asx:
                    abuf = asx.enter_context(tc.tile_pool(name=f"abuf{b}", bufs=1))
                    x_sb = abuf.tile([128, NT, C], f32)
                    ln1T = abuf.tile([128, 6, N], bf16)
                    qkT = abuf.tile([128, 12, N], bf16)   # q o-tiles 0..5, k 6..11
                    v_sb = abuf.tile([128, NT, H, 65], bf16)  # col 64 = ones
                    xatt = abuf.tile([128, 6, N], bf16)   # feature-major attn out

                    # ---- A1: load x, LN1, transpose to feature-major ----
                    with ExitStack() as ph:
                        tp = ph.enter_context(tc.tile_pool(name=f"a1t{b}", bufs=3))
                        pst = ph.enter_context(
                            tc.tile_pool(name=f"a1p{b}", bufs=3, space="PSUM"))
                        for t, (t0, tsz) in enumerate(TOK_TILES):
                            nc.sync.dma_start(out=x_sb[:tsz, t], in_=xs[b, t0:t0 + tsz])
                            layernorm_to_T(tc, tp, pst, x_sb[:tsz, t], tsz, t0,
                                           ln1T, ident, eps_sb)

                    # ---- A2: qkv projections ----
                    with ExitStack() as ph:
                        psqk = ph.enter_context(
                            tc.tile_pool(name=f"a2q{b}", bufs=3, space="PSUM"))
                        psv = ph.enter_context(
                            tc.tile_pool(name=f"a2v{b}", bufs=2, space="PSUM"))
                        # q^T, k^T feature-major [o, tok]
                        for ot in range(12):
                            for (c0, csz) in TOK_CHUNKS:
                                ps = psqk.tile([128, 512], f32, tag="qk")
                                for k in range(6):
                                    nc.tensor.matmul(
                                        ps[:, :csz],
                                        wqk_sb[:, k, ot * 128:(ot + 1) * 128],
                                        ln1T[:, k, c0:c0 + csz],
                                        start=(k == 0), stop=(k == 5))
                                nc.vector.tensor_scalar_add(
                                    qkT[:, ot, c0:c0 + csz], ps[:, :csz],
                                    bqk_sb[:, ot:ot + 1])
                        # v token-major with per-head stride-65 layout + ones col
                        nc.vector.memset(v_sb[:, :, :, 64:65], 1.0)
                        for t, (t0, tsz) in enumerate(TOK_TILES):
                            for (c0, csz) in V_CHUNKS:
                                ps = psv.tile([128, 512], f32, tag="v")
                                for k in range(6):
                                    nc.tensor.matmul(
                                        ps[:tsz, :csz],
                                        ln1T[:, k, t0:t0 + tsz],
                                        wv_sb[:, k, c0:c0 + csz],
                                        start=(k == 0), stop=(k == 5))
                                nc.vector.tensor_add(
                                    v_sb[:tsz, t, c0 // 64:(c0 + csz) // 64, 0:64],
                                    ps[:tsz, :csz].rearrange("p (h d) -> p h d", d=64),
                                    bv_bc[:tsz, c0:c0 + csz].rearrange(
                                        "p (h d) -> p h d", d=64))

                    # ---- A3: attention per head pair ----
                    with ExitStack() as ph:
                        pss = ph.enter_context(
                            tc.tile_pool(name=f"a3s{b}", bufs=1, space="PSUM"))
                        psa = ph.enter_context(
                            tc.tile_pool(name=f"a3a{b}", bufs=1, space="PSUM"))
                        ptp = ph.enter_context(tc.tile_pool(name=f"a3p{b}", bufs=6))
                        dnp = ph.enter_context(tc.tile_pool(name=f"a3d{b}", bufs=3))
                        for hp in range(6):
                            for (kts, qchunks) in ((TMPL_KT, TMPL_QCH),
                                                   (TOK_TILES, SRCH_QCH)):
                                nch = len(qchunks)
                                av = [psa.tile([128, 2, 512], f32, tag=f"av{s}",
                                                name=f"av{s}")
                                      for s in (0, 1)]
                                nkt = len(kts)
                                for ki, (k0, ksz) in enumerate(kts):
                                    kt_t, ko = k0 // 128, k0 % 128
                                    qsz0 = qchunks[0][1]
                                    for s in (0, 1):
                                        pb = s * 64
                                        h = 2 * hp + s
                                        pt = ptp.tile([128, nch, 512], bf16,
                                                      tag=f"pt{s}")
                                        ss = pss.tile([128, 2, 512], f32,
                                                      tag=f"s{s}")
                                        for ci, (q0, qsz) in enumerate(qchunks):
                                            # S^T[kt,qt] = k_h^T.T @ q_h^T (K=64,
                                            # row group s) — pairs run concurrently
                                            nc.tensor.matmul(
                                                ss[:ksz, ci, :qsz],
                                                qkT[pb:pb + 64, 6 + hp, k0:k0 + ksz],
                                                qkT[pb:pb + 64, hp, q0:q0 + qsz],
                                                start=True, stop=True)
                                        # one exp over all chunks of this key tile
                                        nc.scalar.activation(
                                            pt[:ksz, :nch, :qsz0],
                                            ss[:ksz, :nch, :qsz0], AF.Exp)
                                        for ci, (q0, qsz) in enumerate(qchunks):
                                            # unnormalized AV; lhsT col 64 is ones
                                            # -> row 64 of psum = softmax denom
                                            nc.tensor.matmul(
                                                av[s][0:65, ci, :qsz],
                                                v_sb[ko:ko + ksz, kt_t, h, 0:65],
                                                pt[:ksz, ci, :qsz],
                                                start=(ki == 0), stop=(ki == nkt - 1))
                                # normalize: xatt_h = av[0:64] * (1/av[64])
                                for s in (0, 1):
                                    qsz0 = qchunks[0][1]
                                    # copy psum -> SBUF promptly so the next
                                    # head pair's AV matmuls get the banks
                                    avs = dnp.tile([65, 2, 512], f32,
                                                   tag=f"avs{s}")
                                    nc.vector.tensor_copy(
                                        avs[:, :nch, :qsz0],
                                        av[s][0:65, :nch, :qsz0])
                                    bc = dnp.tile([64, 2, 512], f32, tag="bc")
                                    di = ((b * 6 + hp) * 2 + (nch - 1)) * 2 + s
                                    # denom row -> DRAM, reload spread over 49
                                    # partitions so the 8-pass reciprocal runs
                                    # on a short free dim, then bounce back and
                                    # broadcast across 64 partitions (DMA-only)
                                    fr = qsz0 // 49
                                    nc.sync.dma_start(
                                        out=dnb_d[di:di + 1, :nch, :qsz0],
                                        in_=avs[64:65, :nch, :qsz0])
                                    dn_t = dnp.tile([49, 2, 16], f32, tag="dnt")
                                    nc.sync.dma_start(
                                        out=dn_t[:, :nch, :fr],
                                        in_=dnb_d[di, :nch, :qsz0].rearrange(
                                            "a (p f) -> p a f", p=49))
                                    dn_r = dnp.tile([49, 2, 16], f32, tag="dnr")
                                    nc.vector.reciprocal(dn_r[:, :nch, :fr],
                                                         dn_t[:, :nch, :fr])
                                    nc.sync.dma_start(
                                        out=dnb2_d[di, :nch, :qsz0].rearrange(
                                            "a (p f) -> p a f", p=49),
                                        in_=dn_r[:, :nch, :fr])
                                    nc.sync.dma_start(
                                        out=bc[:, :nch, :qsz0],
                                        in_=dnb2_d[di, :nch, :qsz0]
                                        .partition_broadcast(64))
                                    if s == 0:
                                        for ci, (q0, qsz) in enumerate(qchunks):
                                            nc.vector.tensor_mul(
                                                xatt[0:64, hp, q0:q0 + qsz],
                                                avs[0:64, ci, :qsz],
                                                bc[:, ci, :qsz])
                                    else:
                                        stg = dnp.tile([64, 2, 512], bf16, tag="stg")
                                        for ci, (q0, qsz) in enumerate(qchunks):
                                            nc.vector.tensor_mul(
                                                stg[:, ci, :qsz],
                                                avs[0:64, ci, :qsz],
                                                bc[:, ci, :qsz])
                                        for ci, (q0, qsz) in enumerate(qchunks):
                                            # odd head rows live at partitions 64+
                                            nc.sync.dma_start(
                                                out=xatt[64:128, hp, q0:q0 + qsz],
                                                in_=stg[:, ci, :qsz])

                    # ---- A4+A5: proj + residual -> xmid; LN2 ----
                    with ExitStack() as ph:
                        psp = ph.enter_context(
                            tc.tile_pool(name=f"a4p{b}", bufs=3, space="PSUM"))
                        pst = ph.enter_context(
                            tc.tile_pool(name=f"a4t{b}", bufs=3, space="PSUM"))
                        tp = ph.enter_context(tc.tile_pool(name=f"a4s{b}", bufs=3))
                        for t, (t0, tsz) in enumerate(TOK_TILES):
                            xm = tp.tile([128, C], f32, tag="xm")
                            for (c0, csz) in V_CHUNKS:
                                ps = psp.tile([128, 512], f32, tag="pj")
                                for k in range(6):
                                    nc.tensor.matmul(
                                        ps[:tsz, :csz],
                                        xatt[:, k, t0:t0 + tsz],
                                        wproj_sb[:, k, c0:c0 + csz],
                                        start=(k == 0), stop=(k == 5))
                                nc.vector.tensor_add(
                                    xm[:tsz, c0:c0 + csz], ps[:tsz, :csz],
                                    x_sb[:tsz, t, c0:c0 + csz])
                            nc.vector.tensor_add(xm[:tsz], xm[:tsz], bproj_bc[:tsz])
                            nc.sync.dma_start(out=xmid_d[b, t0:t0 + tsz], in_=xm[:tsz])
                            layernorm_to_T(tc, tp, pst, xm[:tsz], tsz, t0,
                                           ln2T, ident, eps_sb)

                # ---- B: MLP (attention buffers released) ----
                with ExitStack() as ph:
                    mw = ph.enter_context(tc.tile_pool(name=f"mw{b}", bufs=1))
                    wf1 = mw.tile([128, 6, HID], bf16)
                    nc.sync.dma_start(out=wf1,
                                      in_=wfc1.rearrange("(k p) o -> p k o", p=128))
                    wf2 = mw.tile([128, 24, C], bf16)
                    nc.sync.dma_start(out=wf2,
                                      in_=wfc2.rearrange("(k p) o -> p k o", p=128))
                    hT = mw.tile([128, 24, N], bf16)
                    ps1 = ph.enter_context(
                        tc.tile_pool(name=f"b1p{b}", bufs=2, space="PSUM"))
                    ps2 = ph.enter_context(
                        tc.tile_pool(name=f"b2p{b}", bufs=3, space="PSUM"))
                    tpm = ph.enter_context(tc.tile_pool(name=f"bt{b}", bufs=3))
                    for ot in range(24):
                        ps = ps1.tile([128, 2, 512], f32, tag="f1")
                        for ci, (c0, csz) in enumerate(TOK_CHUNKS):
                            for k in range(6):
                                nc.tensor.matmul(
                                    ps[:, ci, :csz],
                                    wf1[:, k, ot * 128:(ot + 1) * 128],
                                    ln2T[:, k, c0:c0 + csz],
                                    start=(k == 0), stop=(k == 5))
                        nc.scalar.activation(
                            hT[:, ot, :].rearrange("p (a c) -> p a c", c=490),
                            ps[:, :, :490], AF.Gelu,
                            bias=bfc1_sb[:, ot:ot + 1])
                    for t, (t0, tsz) in enumerate(TOK_TILES):
                        xm = tpm.tile([128, C], f32, tag="xm2")
                        nc.sync.dma_start(out=xm[:tsz], in_=xmid_d[b, t0:t0 + tsz])
                        ot_t = tpm.tile([128, C], f32, tag="ott")
                        for (c0, csz) in V_CHUNKS:
                            ps = ps2.tile([128, 512], f32, tag="f2")
                            for k in range(24):
                                nc.tensor.matmul(
                                    ps[:tsz, :csz],
                                    hT[:, k, t0:t0 + tsz],
                                    wf2[:, k, c0:c0 + csz],
                                    start=(k == 0), stop=(k == 23))
                            nc.vector.tensor_add(
                                ot_t[:tsz, c0:c0 + csz], ps[:tsz, :csz],
                                xm[:tsz, c0:c0 + csz])
                        nc.vector.tensor_add(ot_t[:tsz], ot_t[:tsz], bfc2_bc[:tsz])
                        nc.sync.dma_start(out=out_d[b, t0:t0 + tsz], in_=ot_t[:tsz])

    if hoist:
        _hoist_excess_waits(nc, mybir)
    return nc


def _hoist_excess_waits(nc, mybir, cap=1, nop_cap=1):
    """walrus's 64B instruction encodings fit only ~1 sync-wait command for
    operand-heavy structs (TS/AC/...). Move excess waits onto same-engine
    NoOps inserted right before the instruction."""
    skip = ("InstNoOp", "InstEventSemaphore", "InstCall",
            "InstAllEngineBarrier", "InstUnconditionalBranch", "InstISA")
    n = 0
    for f in nc.m.functions:
        for blk in getattr(f, "blocks", []):
            out = []
            for inst in blk.instructions:
                si = inst.sync_info
                if (si is not None and len(si.on_wait) > cap
                        and type(inst).__name__ not in skip):
                    waits = list(si.on_wait)
                    keep, extra = waits[:cap], waits[cap:]
                    while extra:
                        chunk, extra = extra[:nop_cap], extra[nop_cap:]
                        n += 1
                        out.append(mybir.InstNoOp(
                            name=f"nopw-{n}", engine=inst.engine, ins=[], outs=[],
                            sync_info=mybir.SyncInfo(on_wait=chunk, on_update=[])))
                    inst.sync_info = mybir.SyncInfo(
                        on_wait=keep, on_update=list(si.on_update))
                out.append(inst)
            blk.instructions = out


def _prep_inputs(inputs):
    """Host-side weight folding; returns dict of per-core-constant arrays."""
    import ml_dtypes
    f32 = np.float32
    ln1_w = np.asarray(inputs["ln1_w"], f32)
    ln1_b = np.asarray(inputs["ln1_b"], f32)
    ln2_w = np.asarray(inputs["ln2_w"], f32)
    ln2_b = np.asarray(inputs["ln2_b"], f32)
    qkv_w = np.asarray(inputs["qkv_w"], f32)
    qkv_b = np.asarray(inputs["qkv_b"], f32)
    proj_w = np.asarray(inputs["proj_w"], f32)
    proj_b = np.asarray(inputs["proj_b"], f32)
    fc1_w = np.asarray(inputs["fc1_w"], f32)
    fc1_b = np.asarray(inputs["fc1_b"], f32)
    fc2_w = np.asarray(inputs["fc2_w"], f32)
    fc2_b = np.asarray(inputs["fc2_b"], f32)

    scale = HD ** -0.5
    # fold LN1 affine into qkv; fold attention scale into q
    w_full = ln1_w[:, None] * qkv_w            # [C, 3C]
    b_full = qkv_b + ln1_b @ qkv_w             # [3C]
    w_full = w_full.copy()
    b_full = b_full.copy()
    w_full[:, :C] *= scale
    b_full[:C] *= scale
    wqk = w_full[:, :2 * C]
    bqk = b_full[:2 * C]
    wv = w_full[:, 2 * C:]
    bv = b_full[2 * C:]
    # fold LN2 affine into fc1
    wfc1 = ln2_w[:, None] * fc1_w
    bfc1 = fc1_b + ln2_b @ fc1_w

    bf16 = ml_dtypes.bfloat16
    return {
        "wqk": np.ascontiguousarray(wqk, dtype=bf16),
        "wv": np.ascontiguousarray(wv, dtype=bf16),
        "wproj": np.ascontiguousarray(proj_w, dtype=bf16),
        "wfc1": np.ascontiguousarray(wfc1, dtype=bf16),
        "wfc2": np.ascontiguousarray(fc2_w, dtype=bf16),
        "bqk": np.ascontiguousarray(bqk, dtype=f32),
        "bv": np.ascontiguousarray(bv, dtype=f32),
        "bproj": np.ascontiguousarray(proj_b, dtype=f32),
        "bfc1": np.ascontiguousarray(bfc1, dtype=f32),
        "bfc2": np.ascontiguousarray(fc2_b, dtype=f32),
    }


def _enable_axon_trace():
    """Register the NTFF profile hook that this image's antenv lacks."""
    import types
    from trn_agent_boot.trn_boot import _ntff_profile_via_ctypes
    mod = types.ModuleType("antenv.axon_hooks")
    hook = _ntff_profile_via_ctypes("/opt/axon/libaxon_pjrt.so")
    mod.get_axon_ntff_profile_hook = lambda: hook
    mod.set_axon_ntff_profile_hook = lambda h: None
    sys.modules["antenv.axon_hooks"] = mod
    import concourse.bass_utils as bu
    bu.upload_artifacts = lambda tmpdir: tmpdir  # no artifact bucket here


def _run_on_device(x_full, consts, trace=False):
    """Build + run the SPMD kernel on the 8 cores. Returns (out, exec_ns)."""
    sys.path.insert(0, "/opt/trn_rl_repo")
    from concourse.bass_utils import run_bass_kernel_spmd

    if os.environ.get("BASS_LDW_OPT", "0") == "1":
        import concourse.bass_utils as bu
        if not getattr(bu, "_ldw_patched", False):
            orig_run = bu.run_command

            def _run_ldw(argv, **kw):
                argv = ["--enable-ldw-opt=true"
                        if a == "--enable-ldw-opt=false" else a for a in argv]
                return orig_run(argv, **kw)

            bu.run_command = _run_ldw
            bu._ldw_patched = True

    tmpdir = None
    if trace:
        try:
            _enable_axon_trace()
            tmpdir = os.environ.get("BASS_KERNEL_TRACE_DIR")
            if tmpdir:
                import shutil
                shutil.rmtree(tmpdir, ignore_errors=True)
                os.makedirs(tmpdir, exist_ok=True)
        except Exception as e:  # profiling is best-effort
            print("trace hook setup failed:", e, file=sys.stderr)
            trace = False

    nc = _build_nc()
    in_maps = []
    for core in range(NCORES):
        m = dict(consts)
        m["xs"] = np.ascontiguousarray(
            x_full[core * PER:(core + 1) * PER], dtype=np.float32)
        in_maps.append(m)
    try:
        res = run_bass_kernel_spmd(nc, in_maps, list(range(NCORES)),
                                   trace=trace, tmpdir=tmpdir)
    except Exception:
        if not trace:
            raise
        print("traced run failed; retrying without trace", file=sys.stderr)
        res = run_bass_kernel_spmd(nc, in_maps, list(range(NCORES)), trace=False)
    out = np.concatenate([r["out"] for r in res.results], axis=0)
    return out, res.exec_time_ns


def _subproc_main(tmpdir):
    import ml_dtypes
    data = np.load(os.path.join(tmpdir, "in.npz"))
    consts = {}
    for k in data.files:
        if k == "x":
            continue
        if k.endswith("__bf16"):
            consts[k[:-6]] = data[k].view(ml_dtypes.bfloat16)
        else:
            consts[k] = data[k]
    trace = os.environ.get("BASS_KERNEL_TRACE", "0") == "1"
    out, exec_ns = _run_on_device(data["x"], consts, trace=trace)
    np.savez(os.path.join(tmpdir, "out.npz"), out=out,
             exec_ns=np.int64(exec_ns if exec_ns else -1))


LAST_EXEC_NS = None


def kernel(**inputs):
    global LAST_EXEC_NS
    x = np.asarray(inputs["x"], np.float32)
    assert x.shape == (B, N, C), x.shape
    t_h = int(np.asarray(inputs.get("t_h", 14)))
    t_w = int(np.asarray(inputs.get("t_w", 14)))
    assert t_h * t_w == TLEN, (t_h, t_w)
    consts = _prep_inputs(inputs)

    # Run the device part in a subprocess with a clean JAX platform env, so a
    # harness that pinned JAX_PLATFORMS=cpu (for the reference) doesn't break
    # the PJRT/axon execution path.
    import subprocess
    import tempfile
    with tempfile.TemporaryDirectory() as td:
        saved = {}
        for k, v in consts.items():
            if v.dtype == np.float32:
                saved[k] = v
            else:  # bfloat16 -> ship as uint16 bits
                saved[k + "__bf16"] = v.view(np.uint16)
        np.savez(os.path.join(td, "in.npz"), x=x, **saved)
        env = dict(os.environ)
        env.pop("JAX_PLATFORMS", None)
        pyp = env.get("PYTHONPATH", "")
        here = os.path.dirname(os.path.abspath(__file__))
        env["PYTHONPATH"] = ":".join(p for p in [here, "/opt/trn_rl_repo", pyp] if p)
        subprocess.run(
            [sys.executable, "-c",
             f"import kernel; kernel._subproc_main({td!r})"],
            check=True, env=env)
        data = np.load(os.path.join(td, "out.npz"))
        out = data["out"]
        LAST_EXEC_NS = int(data["exec_ns"])
    return out.astype(np.float32)


if __name__ == "__main__":
    if len(sys.argv) > 1 and sys.argv[1] == "_sub":
        _subproc_main(sys.argv[2])


# revision 34
# speedup vs baseline: 1.1674x; 1.0009x over previous
"""MixFormer block kernel for 8 Trainium2 NeuronCores.

Sharding: data-parallel over batch B=16 -> 2 batch elements per core.
No collectives needed. Each core runs the full block (LN1 -> mixed
attention -> proj residual -> LN2 -> MLP residual) on its 2 batch
elements with bf16 matmuls and fp32 accumulation/residual path.

kernel(**inputs) takes the FULL inputs (as produced by the reference
setup_inputs) and returns the FULL [16, 980, 768] fp32 output.
"""

import os
import sys
import numpy as np

# ---------------------------------------------------------------- constants
B, N, C = 16, 980, 768
H, HD, HID = 12, 64, 3072
TLEN = 196  # t_h * t_w template tokens; search tokens attend to all N
EPS = 1e-5
NCORES = 8
PER = B // NCORES  # batch elements per core

NT = (N + 127) // 128  # 8 token tiles (7x128 + 84)
TOK_TILES = [(i * 128, min(128, N - i * 128)) for i in range(NT)]
TOK_CHUNKS = [(0, 490), (490, 490)]             # moving-dim chunks over tokens
V_CHUNKS = [(0, 512), (512, 256)]               # chunks over C=768 outputs
TMPL_KT = [(0, 128), (128, TLEN - 128)]         # key tiles for template region
TMPL_QCH = [(0, TLEN)]                          # template query chunk
SRCH_QCH = [(TLEN, 392), (TLEN + 392, 392)]     # search query chunks (784 = 2x392)


def _build_nc(hoist=True):
    import concourse.bass as bass
    import concourse.tile as tile
    import concourse.mybir as mybir
    from concourse.masks import make_identity
    from contextlib import ExitStack

    f32 = mybir.dt.float32
    bf16 = mybir.dt.bfloat16
    AF = mybir.ActivationFunctionType
    OP = mybir.AluOpType

    nc = bass.Bass()

    xs = nc.dram_tensor("xs", [PER, N, C], f32, kind="ExternalInput")
    wqk = nc.dram_tensor("wqk", [C, 2 * C], bf16, kind="ExternalInput")
    wv = nc.dram_tensor("wv", [C, C], bf16, kind="ExternalInput")
    wproj = nc.dram_tensor("wproj", [C, C], bf16, kind="ExternalInput")
    wfc1 = nc.dram_tensor("wfc1", [C, HID], bf16, kind="ExternalInput")
    wfc2 = nc.dram_tensor("wfc2", [HID, C], bf16, kind="ExternalInput")
    bqk = nc.dram_tensor("bqk", [2 * C], f32, kind="ExternalInput")
    bv = nc.dram_tensor("bv", [C], f32, kind="ExternalInput")
    bproj = nc.dram_tensor("bproj", [C], f32, kind="ExternalInput")
    bfc1 = nc.dram_tensor("bfc1", [HID], f32, kind="ExternalInput")
    bfc2 = nc.dram_tensor("bfc2", [C], f32, kind="ExternalInput")
    out_d = nc.dram_tensor("out", [PER, N, C], f32, kind="ExternalOutput")
    xmid_d = nc.dram_tensor("xmid", [PER, N, C], f32)  # internal scratch
    dnb_d = nc.dram_tensor("dnb", [48, 2, 512], f32)   # denom bounce buffer
    dnb2_d = nc.dram_tensor("dnb2", [48, 2, 512], f32)  # reciprocal bounce

    def layernorm_to_T(tc, tp, pst, src, tsz, t0, dstT, ident, eps_sb):
        """src: [tsz, 768] fp32 SBUF AP -> dstT[:, :, t0:t0+tsz] feature-major bf16."""
        stats = tp.tile([128, 3, 6], f32, tag="ln_st")
        for g in range(3):
            nc.vector.bn_stats(out=stats[:tsz, g], in_=src[:, g * 256:(g + 1) * 256])
        mv = tp.tile([128, 2], f32, tag="ln_mv")
        nc.vector.bn_aggr(out=mv[:tsz], in_=stats[:tsz])
        # rstd = exp(-0.5 * ln(var + eps)); keeps ACT in the ln/exp table set
        lnv = tp.tile([128, 1], f32, tag="ln_lnv")
        nc.scalar.activation(out=lnv[:tsz], in_=mv[:tsz, 1:2], func=AF.Ln,
                             bias=eps_sb[:tsz])
        rstd = tp.tile([128, 1], f32, tag="ln_rstd")
        nc.scalar.activation(out=rstd[:tsz], in_=lnv[:tsz], func=AF.Exp, scale=-0.5)
        ctr = tp.tile([128, C], f32, tag="ln_ctr")
        nc.vector.tensor_scalar_sub(ctr[:tsz], src, mv[:tsz, 0:1])
        lnt = tp.tile([128, C], bf16, tag="ln_out")
        nc.vector.tensor_scalar_mul(lnt[:tsz], ctr[:tsz], rstd[:tsz, 0:1])
        for c in range(6):
            pt = pst.tile([128, 128], bf16, tag="ln_tr")
            nc.tensor.transpose(pt[:, :tsz], lnt[:tsz, c * 128:(c + 1) * 128],
                                ident[:tsz, :tsz])
            nc.scalar.copy(out=dstT[:, c, t0:t0 + tsz], in_=pt[:, :tsz])

    with tile.TileContext(nc) as tc, ExitStack() as top:
        persist = top.enter_context(tc.tile_pool(name="persist", bufs=1))
        ident = persist.tile([128, 128], bf16)
        make_identity(nc, ident)
        wqk_sb = persist.tile([128, 6, 2 * C], bf16)
        nc.sync.dma_start(out=wqk_sb, in_=wqk.rearrange("(k p) o -> p k o", p=128))
        wv_sb = persist.tile([128, 6, C], bf16)
        nc.sync.dma_start(out=wv_sb, in_=wv.rearrange("(k p) o -> p k o", p=128))
        wproj_sb = persist.tile([128, 6, C], bf16)
        nc.sync.dma_start(out=wproj_sb, in_=wproj.rearrange("(k p) o -> p k o", p=128))
        bqk_sb = persist.tile([128, 12], f32)
        nc.sync.dma_start(out=bqk_sb, in_=bqk.rearrange("(t p) -> p t", p=128))
        bfc1_sb = persist.tile([128, 24], f32)
        nc.sync.dma_start(out=bfc1_sb, in_=bfc1.rearrange("(t p) -> p t", p=128))
        bv_bc = persist.tile([128, C], f32)
        nc.sync.dma_start(out=bv_bc, in_=bv[:].partition_broadcast(128))
        bproj_bc = persist.tile([128, C], f32)
        nc.sync.dma_start(out=bproj_bc, in_=bproj[:].partition_broadcast(128))
        bfc2_bc = persist.tile([128, C], f32)
        nc.sync.dma_start(out=bfc2_bc, in_=bfc2[:].partition_broadcast(128))
        eps_sb = persist.tile([128, 1], f32)
        nc.vector.memset(eps_sb, EPS)

        for b in range(PER):
            with ExitStack() as bs:
                ln2p = bs.enter_context(tc.tile_pool(name=f"ln2p{b}", bufs=1))
                ln2T = ln2p.tile([128, 6, N], bf16)

                with ExitStack() as asx:
                    abuf = asx.enter_context(tc.tile_pool(name=f"abuf{b}", bufs=1))
                    x_sb = abuf.tile([128, NT, C], f32)
                    ln1T = abuf.tile([128, 6, N], bf16)
                    qkT = abuf.tile([128, 12, N], bf16)   # q o-tiles 0..5, k 6..11
                    v_sb = abuf.tile([128, NT, H, 65], bf16)  # col 64 = ones
                    xatt = abuf.tile([128, 6, N], bf16)   # feature-major attn out

                    # ---- A1: load x, LN1, transpose to feature-major ----
                    with ExitStack() as ph:
                        tp = ph.enter_context(tc.tile_pool(name=f"a1t{b}", bufs=6))
                        pst = ph.enter_context(
                            tc.tile_pool(name=f"a1p{b}", bufs=5, space="PSUM"))
                        for t, (t0, tsz) in enumerate(TOK_TILES):
                            nc.sync.dma_start(out=x_sb[:tsz, t], in_=xs[b, t0:t0 + tsz])
                            layernorm_to_T(tc, tp, pst, x_sb[:tsz, t], tsz, t0,
                                           ln1T, ident, eps_sb)

                    # ---- A2: qkv projections ----
                    with ExitStack() as ph:
                        psqk = ph.enter_context(
                            tc.tile_pool(name=f"a2q{b}", bufs=3, space="PSUM"))
                        psv = ph.enter_context(
                            tc.tile_pool(name=f"a2v{b}", bufs=2, space="PSUM"))
                        # q^T, k^T feature-major [o, tok]
                        for ot in range(12):
                            for (c0, csz) in TOK_CHUNKS:
                                ps = psqk.tile([128, 512], f32, tag="qk")
                                for k in range(6):
                                    nc.tensor.matmul(
                                        ps[:, :csz],
                                        wqk_sb[:, k, ot * 128:(ot + 1) * 128],
                                        ln1T[:, k, c0:c0 + csz],
                                        start=(k == 0), stop=(k == 5))
                                nc.vector.tensor_scalar_add(
                                    qkT[:, ot, c0:c0 + csz], ps[:, :csz],
                                    bqk_sb[:, ot:ot + 1])
                        # v token-major with per-head stride-65 layout + ones col
                        nc.vector.memset(v_sb[:, :, :, 64:65], 1.0)
                        for t, (t0, tsz) in enumerate(TOK_TILES):
                            for (c0, csz) in V_CHUNKS:
                                ps = psv.tile([128, 512], f32, tag="v")
                                for k in range(6):
                                    nc.tensor.matmul(
                                        ps[:tsz, :csz],
                                        ln1T[:, k, t0:t0 + tsz],
                                        wv_sb[:, k, c0:c0 + csz],
                                        start=(k == 0), stop=(k == 5))
                                nc.vector.tensor_add(
                                    v_sb[:tsz, t, c0 // 64:(c0 + csz) // 64, 0:64],
                                    ps[:tsz, :csz].rearrange("p (h d) -> p h d", d=64),
                                    bv_bc[:tsz, c0:c0 + csz].rearrange(
                                        "p (h d) -> p h d", d=64))

                    # ---- A3: attention per head pair ----
                    with ExitStack() as ph:
                        pss = ph.enter_context(
                            tc.tile_pool(name=f"a3s{b}", bufs=1, space="PSUM"))
                        psa = ph.enter_context(
                            tc.tile_pool(name=f"a3a{b}", bufs=1, space="PSUM"))
                        ptp = ph.enter_context(tc.tile_pool(name=f"a3p{b}", bufs=6))
                        dnp = ph.enter_context(tc.tile_pool(name=f"a3d{b}", bufs=3))
                        for hp in range(6):
                            for (kts, qchunks) in ((TMPL_KT, TMPL_QCH),
                                                   (TOK_TILES, SRCH_QCH)):
                                nch = len(qchunks)
                                av = [psa.tile([128, 2, 512], f32, tag=f"av{s}",
                                                name=f"av{s}")
                                      for s in (0, 1)]
                                nkt = len(kts)
                                for ki, (k0, ksz) in enumerate(kts):
                                    kt_t, ko = k0 // 128, k0 % 128
                                    qsz0 = qchunks[0][1]
                                    for s in (0, 1):
                                        pb = s * 64
                                        h = 2 * hp + s
                                        pt = ptp.tile([128, nch, 512], bf16,
                                                      tag=f"pt{s}")
                                        ss = pss.tile([128, 2, 512], f32,
                                                      tag=f"s{s}")
                                        for ci, (q0, qsz) in enumerate(qchunks):
                                            # S^T[kt,qt] = k_h^T.T @ q_h^T (K=64,
                                            # row group s) — pairs run concurrently
                                            nc.tensor.matmul(
                                                ss[:ksz, ci, :qsz],
                                                qkT[pb:pb + 64, 6 + hp, k0:k0 + ksz],
                                                qkT[pb:pb + 64, hp, q0:q0 + qsz],
                                                start=True, stop=True)
                                        # one exp over all chunks of this key tile
                                        nc.scalar.activation(
                                            pt[:ksz, :nch, :qsz0],
                                            ss[:ksz, :nch, :qsz0], AF.Exp)
                                        for ci, (q0, qsz) in enumerate(qchunks):
                                            # unnormalized AV; lhsT col 64 is ones
                                            # -> row 64 of psum = softmax denom
                                            nc.tensor.matmul(
                                                av[s][0:65, ci, :qsz],
                                                v_sb[ko:ko + ksz, kt_t, h, 0:65],
                                                pt[:ksz, ci, :qsz],
                                                start=(ki == 0), stop=(ki == nkt - 1))
                                # normalize: xatt_h = av[0:64] * (1/av[64])
                                for s in (0, 1):
                                    qsz0 = qchunks[0][1]
                                    # copy psum -> SBUF promptly so the next
                                    # head pair's AV matmuls get the banks
                                    avs = dnp.tile([65, 2, 512], f32,
                                                   tag=f"avs{s}")
                                    nc.vector.tensor_copy(
                                        avs[:, :nch, :qsz0],
                                        av[s][0:65, :nch, :qsz0])
                                    bc = dnp.tile([64, 2, 512], f32, tag="bc")
                                    di = ((b * 6 + hp) * 2 + (nch - 1)) * 2 + s
                                    # denom row -> DRAM, reload spread over 49
                                    # partitions so the 8-pass reciprocal runs
                                    # on a short free dim, then bounce back and
                                    # broadcast across 64 partitions (DMA-only)
                                    fr = qsz0 // 49
                                    nc.sync.dma_start(
                                        out=dnb_d[di:di + 1, :nch, :qsz0],
                                        in_=avs[64:65, :nch, :qsz0])
                                    dn_t = dnp.tile([49, 2, 16], f32, tag="dnt")
                                    nc.sync.dma_start(
                                        out=dn_t[:, :nch, :fr],
                                        in_=dnb_d[di, :nch, :qsz0].rearrange(
                                            "a (p f) -> p a f", p=49))
                                    dn_r = dnp.tile([49, 2, 16], f32, tag="dnr")
                                    nc.vector.reciprocal(dn_r[:, :nch, :fr],
                                                         dn_t[:, :nch, :fr])
                                    nc.sync.dma_start(
                                        out=dnb2_d[di, :nch, :qsz0].rearrange(
                                            "a (p f) -> p a f", p=49),
                                        in_=dn_r[:, :nch, :fr])
                                    nc.sync.dma_start(
                                        out=bc[:, :nch, :qsz0],
                                        in_=dnb2_d[di, :nch, :qsz0]
                                        .partition_broadcast(64))
                                    if s == 0:
                                        for ci, (q0, qsz) in enumerate(qchunks):
                                            nc.vector.tensor_mul(
                                                xatt[0:64, hp, q0:q0 + qsz],
                                                avs[0:64, ci, :qsz],
                                                bc[:, ci, :qsz])
                                    else:
                                        stg = dnp.tile([64, 2, 512], bf16, tag="stg")
                                        for ci, (q0, qsz) in enumerate(qchunks):
                                            nc.vector.tensor_mul(
                                                stg[:, ci, :qsz],
                                                avs[0:64, ci, :qsz],
                                                bc[:, ci, :qsz])
                                        for ci, (q0, qsz) in enumerate(qchunks):
                                            # odd head rows live at partitions 64+
                                            nc.sync.dma_start(
                                                out=xatt[64:128, hp, q0:q0 + qsz],
                                                in_=stg[:, ci, :qsz])

                    # ---- A4+A5: proj + residual -> xmid; LN2 ----
                    with ExitStack() as ph:
                        psp = ph.enter_context(
                            tc.tile_pool(name=f"a4p{b}", bufs=3, space="PSUM"))
                        pst = ph.enter_context(
                            tc.tile_pool(name=f"a4t{b}", bufs=4, space="PSUM"))
                        tp = ph.enter_context(tc.tile_pool(name=f"a4s{b}", bufs=6))
                        for t, (t0, tsz) in enumerate(TOK_TILES):
                            xm = tp.tile([128, C], f32, tag="xm")
                            for (c0, csz) in V_CHUNKS:
                                ps = psp.tile([128, 512], f32, tag="pj")
                                for k in range(6):
                                    nc.tensor.matmul(
                                        ps[:tsz, :csz],
                                        xatt[:, k, t0:t0 + tsz],
                                        wproj_sb[:, k, c0:c0 + csz],
                                        start=(k == 0), stop=(k == 5))
                                nc.vector.tensor_add(
                                    xm[:tsz, c0:c0 + csz], ps[:tsz, :csz],
                                    x_sb[:tsz, t, c0:c0 + csz])
                            nc.vector.tensor_add(xm[:tsz], xm[:tsz], bproj_bc[:tsz])
                            nc.sync.dma_start(out=xmid_d[b, t0:t0 + tsz], in_=xm[:tsz])
                            layernorm_to_T(tc, tp, pst, xm[:tsz], tsz, t0,
                                           ln2T, ident, eps_sb)

                # ---- B: MLP (attention buffers released) ----
                with ExitStack() as ph:
                    mw = ph.enter_context(tc.tile_pool(name=f"mw{b}", bufs=1))
                    wf1 = mw.tile([128, 6, HID], bf16)
                    nc.sync.dma_start(out=wf1,
                                      in_=wfc1.rearrange("(k p) o -> p k o", p=128))
                    wf2 = mw.tile([128, 24, C], bf16)
                    nc.sync.dma_start(out=wf2,
                                      in_=wfc2.rearrange("(k p) o -> p k o", p=128))
                    hT = mw.tile([128, 24, N], bf16)
                    ps1 = ph.enter_context(
                        tc.tile_pool(name=f"b1p{b}", bufs=2, space="PSUM"))
                    ps2 = ph.enter_context(
                        tc.tile_pool(name=f"b2p{b}", bufs=3, space="PSUM"))
                    tpm = ph.enter_context(tc.tile_pool(name=f"bt{b}", bufs=3))
                    for ot in range(24):
                        ps = ps1.tile([128, 2, 512], f32, tag="f1")
                        for ci, (c0, csz) in enumerate(TOK_CHUNKS):
                            for k in range(6):
                                nc.tensor.matmul(
                                    ps[:, ci, :csz],
                                    wf1[:, k, ot * 128:(ot + 1) * 128],
                                    ln2T[:, k, c0:c0 + csz],
                                    start=(k == 0), stop=(k == 5))
                        nc.scalar.activation(
                            hT[:, ot, :].rearrange("p (a c) -> p a c", c=490),
                            ps[:, :, :490], AF.Gelu,
                            bias=bfc1_sb[:, ot:ot + 1])
                    for t, (t0, tsz) in enumerate(TOK_TILES):
                        xm = tpm.tile([128, C], f32, tag="xm2")
                        nc.sync.dma_start(out=xm[:tsz], in_=xmid_d[b, t0:t0 + tsz])
                        ot_t = tpm.tile([128, C], f32, tag="ott")
                        for (c0, csz) in V_CHUNKS:
                            ps = ps2.tile([128, 512], f32, tag="f2")
                            for k in range(24):
                                nc.tensor.matmul(
                                    ps[:tsz, :csz],
                                    hT[:, k, t0:t0 + tsz],
                                    wf2[:, k, c0:c0 + csz],
                                    start=(k == 0), stop=(k == 23))
                            nc.vector.tensor_add(
                                ot_t[:tsz, c0:c0 + csz], ps[:tsz, :csz],
                                xm[:tsz, c0:c0 + csz])
                        nc.vector.tensor_add(ot_t[:tsz], ot_t[:tsz], bfc2_bc[:tsz])
                        nc.sync.dma_start(out=out_d[b, t0:t0 + tsz], in_=ot_t[:tsz])

    if hoist:
        _hoist_excess_waits(nc, mybir)
    return nc


def _hoist_excess_waits(nc, mybir, cap=1, nop_cap=1):
    """walrus's 64B instruction encodings fit only ~1 sync-wait command for
    operand-heavy structs (TS/AC/...). Move excess waits onto same-engine
    NoOps inserted right before the instruction."""
    skip = ("InstNoOp", "InstEventSemaphore", "InstCall",
            "InstAllEngineBarrier", "InstUnconditionalBranch", "InstISA")
    n = 0
    for f in nc.m.functions:
        for blk in getattr(f, "blocks", []):
            out = []
            for inst in blk.instructions:
                si = inst.sync_info
                if (si is not None and len(si.on_wait) > cap
                        and type(inst).__name__ not in skip):
                    waits = list(si.on_wait)
                    keep, extra = waits[:cap], waits[cap:]
                    while extra:
                        chunk, extra = extra[:nop_cap], extra[nop_cap:]
                        n += 1
                        out.append(mybir.InstNoOp(
                            name=f"nopw-{n}", engine=inst.engine, ins=[], outs=[],
                            sync_info=mybir.SyncInfo(on_wait=chunk, on_update=[])))
                    inst.sync_info = mybir.SyncInfo(
                        on_wait=keep, on_update=list(si.on_update))
                out.append(inst)
            blk.instructions = out


def _prep_inputs(inputs):
    """Host-side weight folding; returns dict of per-core-constant arrays."""
    import ml_dtypes
    f32 = np.float32
    ln1_w = np.asarray(inputs["ln1_w"], f32)
    ln1_b = np.asarray(inputs["ln1_b"], f32)
    ln2_w = np.asarray(inputs["ln2_w"], f32)
    ln2_b = np.asarray(inputs["ln2_b"], f32)
    qkv_w = np.asarray(inputs["qkv_w"], f32)
    qkv_b = np.asarray(inputs["qkv_b"], f32)
    proj_w = np.asarray(inputs["proj_w"], f32)
    proj_b = np.asarray(inputs["proj_b"], f32)
    fc1_w = np.asarray(inputs["fc1_w"], f32)
    fc1_b = np.asarray(inputs["fc1_b"], f32)
    fc2_w = np.asarray(inputs["fc2_w"], f32)
    fc2_b = np.asarray(inputs["fc2_b"], f32)

    scale = HD ** -0.5
    # fold LN1 affine into qkv; fold attention scale into q
    w_full = ln1_w[:, None] * qkv_w            # [C, 3C]
    b_full = qkv_b + ln1_b @ qkv_w             # [3C]
    w_full = w_full.copy()
    b_full = b_full.copy()
    w_full[:, :C] *= scale
    b_full[:C] *= scale
    wqk = w_full[:, :2 * C]
    bqk = b_full[:2 * C]
    wv = w_full[:, 2 * C:]
    bv = b_full[2 * C:]
    # fold LN2 affine into fc1
    wfc1 = ln2_w[:, None] * fc1_w
    bfc1 = fc1_b + ln2_b @ fc1_w

    bf16 = ml_dtypes.bfloat16
    return {
        "wqk": np.ascontiguousarray(wqk, dtype=bf16),
        "wv": np.ascontiguousarray(wv, dtype=bf16),
        "wproj": np.ascontiguousarray(proj_w, dtype=bf16),
        "wfc1": np.ascontiguousarray(wfc1, dtype=bf16),
        "wfc2": np.ascontiguousarray(fc2_w, dtype=bf16),
        "bqk": np.ascontiguousarray(bqk, dtype=f32),
        "bv": np.ascontiguousarray(bv, dtype=f32),
        "bproj": np.ascontiguousarray(proj_b, dtype=f32),
        "bfc1": np.ascontiguousarray(bfc1, dtype=f32),
        "bfc2": np.ascontiguousarray(fc2_b, dtype=f32),
    }


def _enable_axon_trace():
    """Register the NTFF profile hook that this image's antenv lacks."""
    import types
    from trn_agent_boot.trn_boot import _ntff_profile_via_ctypes
    mod = types.ModuleType("antenv.axon_hooks")
    hook = _ntff_profile_via_ctypes("/opt/axon/libaxon_pjrt.so")
    mod.get_axon_ntff_profile_hook = lambda: hook
    mod.set_axon_ntff_profile_hook = lambda h: None
    sys.modules["antenv.axon_hooks"] = mod
    import concourse.bass_utils as bu
    bu.upload_artifacts = lambda tmpdir: tmpdir  # no artifact bucket here


def _run_on_device(x_full, consts, trace=False):
    """Build + run the SPMD kernel on the 8 cores. Returns (out, exec_ns)."""
    sys.path.insert(0, "/opt/trn_rl_repo")
    from concourse.bass_utils import run_bass_kernel_spmd

    if os.environ.get("BASS_LDW_OPT", "0") == "1":
        import concourse.bass_utils as bu
        if not getattr(bu, "_ldw_patched", False):
            orig_run = bu.run_command

            def _run_ldw(argv, **kw):
                argv = ["--enable-ldw-opt=true"
                        if a == "--enable-ldw-opt=false" else a for a in argv]
                return orig_run(argv, **kw)

            bu.run_command = _run_ldw
            bu._ldw_patched = True

    tmpdir = None
    if trace:
        try:
            _enable_axon_trace()
            tmpdir = os.environ.get("BASS_KERNEL_TRACE_DIR")
            if tmpdir:
                import shutil
                shutil.rmtree(tmpdir, ignore_errors=True)
                os.makedirs(tmpdir, exist_ok=True)
        except Exception as e:  # profiling is best-effort
            print("trace hook setup failed:", e, file=sys.stderr)
            trace = False

    nc = _build_nc()
    in_maps = []
    for core in range(NCORES):
        m = dict(consts)
        m["xs"] = np.ascontiguousarray(
            x_full[core * PER:(core + 1) * PER], dtype=np.float32)
        in_maps.append(m)
    try:
        res = run_bass_kernel_spmd(nc, in_maps, list(range(NCORES)),
                                   trace=trace, tmpdir=tmpdir)
    except Exception:
        if not trace:
            raise
        print("traced run failed; retrying without trace", file=sys.stderr)
        res = run_bass_kernel_spmd(nc, in_maps, list(range(NCORES)), trace=False)
    out = np.concatenate([r["out"] for r in res.results], axis=0)
    return out, res.exec_time_ns


def _subproc_main(tmpdir):
    import ml_dtypes
    data = np.load(os.path.join(tmpdir, "in.npz"))
    consts = {}
    for k in data.files:
        if k == "x":
            continue
        if k.endswith("__bf16"):
            consts[k[:-6]] = data[k].view(ml_dtypes.bfloat16)
        else:
            consts[k] = data[k]
    trace = os.environ.get("BASS_KERNEL_TRACE", "0") == "1"
    out, exec_ns = _run_on_device(data["x"], consts, trace=trace)
    np.savez(os.path.join(tmpdir, "out.npz"), out=out,
             exec_ns=np.int64(exec_ns if exec_ns else -1))


LAST_EXEC_NS = None


def kernel(**inputs):
    global LAST_EXEC_NS
    x = np.asarray(inputs["x"], np.float32)
    assert x.shape == (B, N, C), x.shape
    t_h = int(np.asarray(inputs.get("t_h", 14)))
    t_w = int(np.asarray(inputs.get("t_w", 14)))
    assert t_h * t_w == TLEN, (t_h, t_w)
    consts = _prep_inputs(inputs)

    # Run the device part in a subprocess with a clean JAX platform env, so a
    # harness that pinned JAX_PLATFORMS=cpu (for the reference) doesn't break
    # the PJRT/axon execution path.
    import subprocess
    import tempfile
    with tempfile.TemporaryDirectory() as td:
        saved = {}
        for k, v in consts.items():
            if v.dtype == np.float32:
                saved[k] = v
            else:  # bfloat16 -> ship as uint16 bits
                saved[k + "__bf16"] = v.view(np.uint16)
        np.savez(os.path.join(td, "in.npz"), x=x, **saved)
        env = dict(os.environ)
        env.pop("JAX_PLATFORMS", None)
        pyp = env.get("PYTHONPATH", "")
        here = os.path.dirname(os.path.abspath(__file__))
        env["PYTHONPATH"] = ":".join(p for p in [here, "/opt/trn_rl_repo", pyp] if p)
        subprocess.run(
            [sys.executable, "-c",
             f"import kernel; kernel._subproc_main({td!r})"],
            check=True, env=env)
        data = np.load(os.path.join(td, "out.npz"))
        out = data["out"]
        LAST_EXEC_NS = int(data["exec_ns"])
    return out.astype(np.float32)


if __name__ == "__main__":
    if len(sys.argv) > 1 and sys.argv[1] == "_sub":
        _subproc_main(sys.argv[2])


# revision 35
# speedup vs baseline: 1.1865x; 1.0163x over previous
"""MixFormer block kernel for 8 Trainium2 NeuronCores.

Sharding: data-parallel over batch B=16 -> 2 batch elements per core.
No collectives needed. Each core runs the full block (LN1 -> mixed
attention -> proj residual -> LN2 -> MLP residual) on its 2 batch
elements with bf16 matmuls and fp32 accumulation/residual path.

kernel(**inputs) takes the FULL inputs (as produced by the reference
setup_inputs) and returns the FULL [16, 980, 768] fp32 output.
"""

import os
import sys
import numpy as np

# ---------------------------------------------------------------- constants
B, N, C = 16, 980, 768
H, HD, HID = 12, 64, 3072
TLEN = 196  # t_h * t_w template tokens; search tokens attend to all N
EPS = 1e-5
NCORES = 8
PER = B // NCORES  # batch elements per core

NT = (N + 127) // 128  # 8 token tiles (7x128 + 84)
TOK_TILES = [(i * 128, min(128, N - i * 128)) for i in range(NT)]
TOK_CHUNKS = [(0, 490), (490, 490)]             # moving-dim chunks over tokens
V_CHUNKS = [(0, 512), (512, 256)]               # chunks over C=768 outputs
TMPL_KT = [(0, 128), (128, TLEN - 128)]         # key tiles for template region
TMPL_QCH = [(0, TLEN)]                          # template query chunk
SRCH_QCH = [(TLEN, 392), (TLEN + 392, 392)]     # search query chunks (784 = 2x392)


def _build_nc(hoist=True):
    import concourse.bass as bass
    import concourse.tile as tile
    import concourse.mybir as mybir
    from concourse.masks import make_identity
    from contextlib import ExitStack

    f32 = mybir.dt.float32
    bf16 = mybir.dt.bfloat16
    AF = mybir.ActivationFunctionType
    OP = mybir.AluOpType

    nc = bass.Bass()

    xs = nc.dram_tensor("xs", [PER, N, C], f32, kind="ExternalInput")
    wqk = nc.dram_tensor("wqk", [C, 2 * C], bf16, kind="ExternalInput")
    wv = nc.dram_tensor("wv", [C, C], bf16, kind="ExternalInput")
    wproj = nc.dram_tensor("wproj", [C, C], bf16, kind="ExternalInput")
    wfc1 = nc.dram_tensor("wfc1", [C, HID], bf16, kind="ExternalInput")
    wfc2 = nc.dram_tensor("wfc2", [HID, C], bf16, kind="ExternalInput")
    bqk = nc.dram_tensor("bqk", [2 * C], f32, kind="ExternalInput")
    bv = nc.dram_tensor("bv", [C], f32, kind="ExternalInput")
    bproj = nc.dram_tensor("bproj", [C], f32, kind="ExternalInput")
    bfc1 = nc.dram_tensor("bfc1", [HID], f32, kind="ExternalInput")
    bfc2 = nc.dram_tensor("bfc2", [C], f32, kind="ExternalInput")
    out_d = nc.dram_tensor("out", [PER, N, C], f32, kind="ExternalOutput")
    xmid_d = nc.dram_tensor("xmid", [PER, N, C], f32)  # internal scratch
    dnb_d = nc.dram_tensor("dnb", [48, 2, 512], f32)   # denom bounce buffer
    dnb2_d = nc.dram_tensor("dnb2", [48, 2, 512], f32)  # reciprocal bounce

    def layernorm_to_T(tc, tp, pst, src, tsz, t0, dstT, ident, eps_sb):
        """src: [tsz, 768] fp32 SBUF AP -> dstT[:, :, t0:t0+tsz] feature-major bf16."""
        stats = tp.tile([128, 3, 6], f32, tag="ln_st")
        for g in range(3):
            nc.vector.bn_stats(out=stats[:tsz, g], in_=src[:, g * 256:(g + 1) * 256])
        mv = tp.tile([128, 2], f32, tag="ln_mv")
        nc.vector.bn_aggr(out=mv[:tsz], in_=stats[:tsz])
        # rstd = exp(-0.5 * ln(var + eps)); keeps ACT in the ln/exp table set
        lnv = tp.tile([128, 1], f32, tag="ln_lnv")
        nc.scalar.activation(out=lnv[:tsz], in_=mv[:tsz, 1:2], func=AF.Ln,
                             bias=eps_sb[:tsz])
        rstd = tp.tile([128, 1], f32, tag="ln_rstd")
        nc.scalar.activation(out=rstd[:tsz], in_=lnv[:tsz], func=AF.Exp, scale=-0.5)
        ctr = tp.tile([128, C], f32, tag="ln_ctr")
        nc.vector.tensor_scalar_sub(ctr[:tsz], src, mv[:tsz, 0:1])
        lnt = tp.tile([128, C], bf16, tag="ln_out")
        nc.vector.tensor_scalar_mul(lnt[:tsz], ctr[:tsz], rstd[:tsz, 0:1])
        for c in range(6):
            pt = pst.tile([128, 128], bf16, tag="ln_tr")
            nc.tensor.transpose(pt[:, :tsz], lnt[:tsz, c * 128:(c + 1) * 128],
                                ident[:tsz, :tsz])
            nc.scalar.copy(out=dstT[:, c, t0:t0 + tsz], in_=pt[:, :tsz])

    with tile.TileContext(nc) as tc, ExitStack() as top:
        persist = top.enter_context(tc.tile_pool(name="persist", bufs=1))
        ident = persist.tile([128, 128], bf16)
        make_identity(nc, ident)
        wqk_sb = persist.tile([128, 6, 2 * C], bf16)
        nc.sync.dma_start(out=wqk_sb, in_=wqk.rearrange("(k p) o -> p k o", p=128))
        wv_sb = persist.tile([128, 6, C], bf16)
        nc.sync.dma_start(out=wv_sb, in_=wv.rearrange("(k p) o -> p k o", p=128))
        wproj_sb = persist.tile([128, 6, C], bf16)
        nc.sync.dma_start(out=wproj_sb, in_=wproj.rearrange("(k p) o -> p k o", p=128))
        bqk_sb = persist.tile([128, 12], f32)
        nc.sync.dma_start(out=bqk_sb, in_=bqk.rearrange("(t p) -> p t", p=128))
        bfc1_sb = persist.tile([128, 24], f32)
        nc.sync.dma_start(out=bfc1_sb, in_=bfc1.rearrange("(t p) -> p t", p=128))
        bv_bc = persist.tile([128, C], f32)
        nc.sync.dma_start(out=bv_bc, in_=bv[:].partition_broadcast(128))
        bproj_bc = persist.tile([128, C], f32)
        nc.sync.dma_start(out=bproj_bc, in_=bproj[:].partition_broadcast(128))
        bfc2_bc = persist.tile([128, C], f32)
        nc.sync.dma_start(out=bfc2_bc, in_=bfc2[:].partition_broadcast(128))
        eps_sb = persist.tile([128, 1], f32)
        nc.vector.memset(eps_sb, EPS)

        for b in range(PER):
            with ExitStack() as bs:
                ln2p = bs.enter_context(tc.tile_pool(name=f"ln2p{b}", bufs=1))
                ln2T = ln2p.tile([128, 6, N], bf16)

                with ExitStack() as asx:
                    abuf = asx.enter_context(tc.tile_pool(name=f"abuf{b}", bufs=1))
                    x_sb = abuf.tile([128, NT, C], f32)
                    ln1T = abuf.tile([128, 6, N], bf16)
                    qkT = abuf.tile([128, 12, N], bf16)   # q o-tiles 0..5, k 6..11
                    v_sb = abuf.tile([128, NT, H, 65], bf16)  # col 64 = ones
                    xatt = abuf.tile([128, 6, N], bf16)   # feature-major attn out

                    # ---- A1: load x, LN1, transpose to feature-major ----
                    with ExitStack() as ph:
                        tp = ph.enter_context(tc.tile_pool(name=f"a1t{b}", bufs=6))
                        pst = ph.enter_context(
                            tc.tile_pool(name=f"a1p{b}", bufs=5, space="PSUM"))
                        for t, (t0, tsz) in enumerate(TOK_TILES):
                            nc.sync.dma_start(out=x_sb[:tsz, t], in_=xs[b, t0:t0 + tsz])
                            layernorm_to_T(tc, tp, pst, x_sb[:tsz, t], tsz, t0,
                                           ln1T, ident, eps_sb)

                    # ---- A2: qkv projections ----
                    with ExitStack() as ph:
                        psqk = ph.enter_context(
                            tc.tile_pool(name=f"a2q{b}", bufs=4, space="PSUM"))
                        psv = ph.enter_context(
                            tc.tile_pool(name=f"a2v{b}", bufs=3, space="PSUM"))
                        # q^T, k^T feature-major [o, tok]
                        for ot in range(12):
                            for (c0, csz) in TOK_CHUNKS:
                                ps = psqk.tile([128, 512], f32, tag="qk")
                                for k in range(6):
                                    nc.tensor.matmul(
                                        ps[:, :csz],
                                        wqk_sb[:, k, ot * 128:(ot + 1) * 128],
                                        ln1T[:, k, c0:c0 + csz],
                                        start=(k == 0), stop=(k == 5))
                                nc.vector.tensor_scalar_add(
                                    qkT[:, ot, c0:c0 + csz], ps[:, :csz],
                                    bqk_sb[:, ot:ot + 1])
                        # v token-major with per-head stride-65 layout + ones col
                        nc.vector.memset(v_sb[:, :, :, 64:65], 1.0)
                        for t, (t0, tsz) in enumerate(TOK_TILES):
                            for (c0, csz) in V_CHUNKS:
                                ps = psv.tile([128, 512], f32, tag="v")
                                for k in range(6):
                                    nc.tensor.matmul(
                                        ps[:tsz, :csz],
                                        ln1T[:, k, t0:t0 + tsz],
                                        wv_sb[:, k, c0:c0 + csz],
                                        start=(k == 0), stop=(k == 5))
                                nc.vector.tensor_add(
                                    v_sb[:tsz, t, c0 // 64:(c0 + csz) // 64, 0:64],
                                    ps[:tsz, :csz].rearrange("p (h d) -> p h d", d=64),
                                    bv_bc[:tsz, c0:c0 + csz].rearrange(
                                        "p (h d) -> p h d", d=64))

                    # ---- A3: attention per head pair ----
                    with ExitStack() as ph:
                        pss = ph.enter_context(
                            tc.tile_pool(name=f"a3s{b}", bufs=1, space="PSUM"))
                        psa = ph.enter_context(
                            tc.tile_pool(name=f"a3a{b}", bufs=1, space="PSUM"))
                        ptp = ph.enter_context(tc.tile_pool(name=f"a3p{b}", bufs=6))
                        dnp = ph.enter_context(tc.tile_pool(name=f"a3d{b}", bufs=3))
                        for hp in range(6):
                            for (kts, qchunks) in ((TMPL_KT, TMPL_QCH),
                                                   (TOK_TILES, SRCH_QCH)):
                                nch = len(qchunks)
                                av = [psa.tile([128, 2, 512], f32, tag=f"av{s}",
                                                name=f"av{s}")
                                      for s in (0, 1)]
                                nkt = len(kts)
                                for ki, (k0, ksz) in enumerate(kts):
                                    kt_t, ko = k0 // 128, k0 % 128
                                    qsz0 = qchunks[0][1]
                                    for s in (0, 1):
                                        pb = s * 64
                                        h = 2 * hp + s
                                        pt = ptp.tile([128, nch, 512], bf16,
                                                      tag=f"pt{s}")
                                        ss = pss.tile([128, 2, 512], f32,
                                                      tag=f"s{s}")
                                        for ci, (q0, qsz) in enumerate(qchunks):
                                            # S^T[kt,qt] = k_h^T.T @ q_h^T (K=64,
                                            # row group s) — pairs run concurrently
                                            nc.tensor.matmul(
                                                ss[:ksz, ci, :qsz],
                                                qkT[pb:pb + 64, 6 + hp, k0:k0 + ksz],
                                                qkT[pb:pb + 64, hp, q0:q0 + qsz],
                                                start=True, stop=True)
                                        # one exp over all chunks of this key tile
                                        nc.scalar.activation(
                                            pt[:ksz, :nch, :qsz0],
                                            ss[:ksz, :nch, :qsz0], AF.Exp)
                                        for ci, (q0, qsz) in enumerate(qchunks):
                                            # unnormalized AV; lhsT col 64 is ones
                                            # -> row 64 of psum = softmax denom
                                            nc.tensor.matmul(
                                                av[s][0:65, ci, :qsz],
                                                v_sb[ko:ko + ksz, kt_t, h, 0:65],
                                                pt[:ksz, ci, :qsz],
                                                start=(ki == 0), stop=(ki == nkt - 1))
                                # normalize: xatt_h = av[0:64] * (1/av[64])
                                for s in (0, 1):
                                    qsz0 = qchunks[0][1]
                                    # copy psum -> SBUF promptly so the next
                                    # head pair's AV matmuls get the banks
                                    avs = dnp.tile([65, 2, 512], f32,
                                                   tag=f"avs{s}")
                                    nc.vector.tensor_copy(
                                        avs[:, :nch, :qsz0],
                                        av[s][0:65, :nch, :qsz0])
                                    bc = dnp.tile([64, 2, 512], f32, tag="bc")
                                    di = ((b * 6 + hp) * 2 + (nch - 1)) * 2 + s
                                    # denom row -> DRAM, reload spread over 49
                                    # partitions so the 8-pass reciprocal runs
                                    # on a short free dim, then bounce back and
                                    # broadcast across 64 partitions (DMA-only)
                                    fr = qsz0 // 49
                                    nc.sync.dma_start(
                                        out=dnb_d[di:di + 1, :nch, :qsz0],
                                        in_=avs[64:65, :nch, :qsz0])
                                    dn_t = dnp.tile([49, 2, 16], f32, tag="dnt")
                                    nc.sync.dma_start(
                                        out=dn_t[:, :nch, :fr],
                                        in_=dnb_d[di, :nch, :qsz0].rearrange(
                                            "a (p f) -> p a f", p=49))
                                    dn_r = dnp.tile([49, 2, 16], f32, tag="dnr")
                                    nc.vector.reciprocal(dn_r[:, :nch, :fr],
                                                         dn_t[:, :nch, :fr])
                                    nc.sync.dma_start(
                                        out=dnb2_d[di, :nch, :qsz0].rearrange(
                                            "a (p f) -> p a f", p=49),
                                        in_=dn_r[:, :nch, :fr])
                                    nc.sync.dma_start(
                                        out=bc[:, :nch, :qsz0],
                                        in_=dnb2_d[di, :nch, :qsz0]
                                        .partition_broadcast(64))
                                    if s == 0:
                                        for ci, (q0, qsz) in enumerate(qchunks):
                                            nc.vector.tensor_mul(
                                                xatt[0:64, hp, q0:q0 + qsz],
                                                avs[0:64, ci, :qsz],
                                                bc[:, ci, :qsz])
                                    else:
                                        stg = dnp.tile([64, 2, 512], bf16, tag="stg")
                                        for ci, (q0, qsz) in enumerate(qchunks):
                                            nc.vector.tensor_mul(
                                                stg[:, ci, :qsz],
                                                avs[0:64, ci, :qsz],
                                                bc[:, ci, :qsz])
                                        for ci, (q0, qsz) in enumerate(qchunks):
                                            # odd head rows live at partitions 64+
                                            nc.sync.dma_start(
                                                out=xatt[64:128, hp, q0:q0 + qsz],
                                                in_=stg[:, ci, :qsz])

                    # ---- A4+A5: proj + residual -> xmid; LN2 ----
                    with ExitStack() as ph:
                        psp = ph.enter_context(
                            tc.tile_pool(name=f"a4p{b}", bufs=3, space="PSUM"))
                        pst = ph.enter_context(
                            tc.tile_pool(name=f"a4t{b}", bufs=4, space="PSUM"))
                        tp = ph.enter_context(tc.tile_pool(name=f"a4s{b}", bufs=6))
                        for t, (t0, tsz) in enumerate(TOK_TILES):
                            xm = tp.tile([128, C], f32, tag="xm")
                            for (c0, csz) in V_CHUNKS:
                                ps = psp.tile([128, 512], f32, tag="pj")
                                for k in range(6):
                                    nc.tensor.matmul(
                                        ps[:tsz, :csz],
                                        xatt[:, k, t0:t0 + tsz],
                                        wproj_sb[:, k, c0:c0 + csz],
                                        start=(k == 0), stop=(k == 5))
                                nc.vector.tensor_add(
                                    xm[:tsz, c0:c0 + csz], ps[:tsz, :csz],
                                    x_sb[:tsz, t, c0:c0 + csz])
                            nc.vector.tensor_add(xm[:tsz], xm[:tsz], bproj_bc[:tsz])
                            nc.sync.dma_start(out=xmid_d[b, t0:t0 + tsz], in_=xm[:tsz])
                            layernorm_to_T(tc, tp, pst, xm[:tsz], tsz, t0,
                                           ln2T, ident, eps_sb)

                # ---- B: MLP (attention buffers released) ----
                with ExitStack() as ph:
                    mw = ph.enter_context(tc.tile_pool(name=f"mw{b}", bufs=1))
                    wf1 = mw.tile([128, 6, HID], bf16)
                    nc.sync.dma_start(out=wf1,
                                      in_=wfc1.rearrange("(k p) o -> p k o", p=128))
                    wf2 = mw.tile([128, 24, C], bf16)
                    nc.sync.dma_start(out=wf2,
                                      in_=wfc2.rearrange("(k p) o -> p k o", p=128))
                    hT = mw.tile([128, 24, N], bf16)
                    ps1 = ph.enter_context(
                        tc.tile_pool(name=f"b1p{b}", bufs=2, space="PSUM"))
                    ps2 = ph.enter_context(
                        tc.tile_pool(name=f"b2p{b}", bufs=4, space="PSUM"))
                    tpm = ph.enter_context(tc.tile_pool(name=f"bt{b}", bufs=3))
                    for ot in range(24):
                        ps = ps1.tile([128, 2, 512], f32, tag="f1")
                        for ci, (c0, csz) in enumerate(TOK_CHUNKS):
                            for k in range(6):
                                nc.tensor.matmul(
                                    ps[:, ci, :csz],
                                    wf1[:, k, ot * 128:(ot + 1) * 128],
                                    ln2T[:, k, c0:c0 + csz],
                                    start=(k == 0), stop=(k == 5))
                        nc.scalar.activation(
                            hT[:, ot, :].rearrange("p (a c) -> p a c", c=490),
                            ps[:, :, :490], AF.Gelu,
                            bias=bfc1_sb[:, ot:ot + 1])
                    for t, (t0, tsz) in enumerate(TOK_TILES):
                        xm = tpm.tile([128, C], f32, tag="xm2")
                        nc.sync.dma_start(out=xm[:tsz], in_=xmid_d[b, t0:t0 + tsz])
                        ot_t = tpm.tile([128, C], f32, tag="ott")
                        for (c0, csz) in V_CHUNKS:
                            ps = ps2.tile([128, 512], f32, tag="f2")
                            for k in range(24):
                                nc.tensor.matmul(
                                    ps[:tsz, :csz],
                                    hT[:, k, t0:t0 + tsz],
                                    wf2[:, k, c0:c0 + csz],
                                    start=(k == 0), stop=(k == 23))
                            nc.vector.tensor_add(
                                ot_t[:tsz, c0:c0 + csz], ps[:tsz, :csz],
                                xm[:tsz, c0:c0 + csz])
                        nc.vector.tensor_add(ot_t[:tsz], ot_t[:tsz], bfc2_bc[:tsz])
                        nc.sync.dma_start(out=out_d[b, t0:t0 + tsz], in_=ot_t[:tsz])

    if hoist:
        _hoist_excess_waits(nc, mybir)
    return nc


def _hoist_excess_waits(nc, mybir, cap=1, nop_cap=1):
    """walrus's 64B instruction encodings fit only ~1 sync-wait command for
    operand-heavy structs (TS/AC/...). Move excess waits onto same-engine
    NoOps inserted right before the instruction."""
    skip = ("InstNoOp", "InstEventSemaphore", "InstCall",
            "InstAllEngineBarrier", "InstUnconditionalBranch", "InstISA")
    n = 0
    for f in nc.m.functions:
        for blk in getattr(f, "blocks", []):
            out = []
            for inst in blk.instructions:
                si = inst.sync_info
                if (si is not None and len(si.on_wait) > cap
                        and type(inst).__name__ not in skip):
                    waits = list(si.on_wait)
                    keep, extra = waits[:cap], waits[cap:]
                    while extra:
                        chunk, extra = extra[:nop_cap], extra[nop_cap:]
                        n += 1
                        out.append(mybir.InstNoOp(
                            name=f"nopw-{n}", engine=inst.engine, ins=[], outs=[],
                            sync_info=mybir.SyncInfo(on_wait=chunk, on_update=[])))
                    inst.sync_info = mybir.SyncInfo(
                        on_wait=keep, on_update=list(si.on_update))
                out.append(inst)
            blk.instructions = out


def _prep_inputs(inputs):
    """Host-side weight folding; returns dict of per-core-constant arrays."""
    import ml_dtypes
    f32 = np.float32
    ln1_w = np.asarray(inputs["ln1_w"], f32)
    ln1_b = np.asarray(inputs["ln1_b"], f32)
    ln2_w = np.asarray(inputs["ln2_w"], f32)
    ln2_b = np.asarray(inputs["ln2_b"], f32)
    qkv_w = np.asarray(inputs["qkv_w"], f32)
    qkv_b = np.asarray(inputs["qkv_b"], f32)
    proj_w = np.asarray(inputs["proj_w"], f32)
    proj_b = np.asarray(inputs["proj_b"], f32)
    fc1_w = np.asarray(inputs["fc1_w"], f32)
    fc1_b = np.asarray(inputs["fc1_b"], f32)
    fc2_w = np.asarray(inputs["fc2_w"], f32)
    fc2_b = np.asarray(inputs["fc2_b"], f32)

    scale = HD ** -0.5
    # fold LN1 affine into qkv; fold attention scale into q
    w_full = ln1_w[:, None] * qkv_w            # [C, 3C]
    b_full = qkv_b + ln1_b @ qkv_w             # [3C]
    w_full = w_full.copy()
    b_full = b_full.copy()
    w_full[:, :C] *= scale
    b_full[:C] *= scale
    wqk = w_full[:, :2 * C]
    bqk = b_full[:2 * C]
    wv = w_full[:, 2 * C:]
    bv = b_full[2 * C:]
    # fold LN2 affine into fc1
    wfc1 = ln2_w[:, None] * fc1_w
    bfc1 = fc1_b + ln2_b @ fc1_w

    bf16 = ml_dtypes.bfloat16
    return {
        "wqk": np.ascontiguousarray(wqk, dtype=bf16),
        "wv": np.ascontiguousarray(wv, dtype=bf16),
        "wproj": np.ascontiguousarray(proj_w, dtype=bf16),
        "wfc1": np.ascontiguousarray(wfc1, dtype=bf16),
        "wfc2": np.ascontiguousarray(fc2_w, dtype=bf16),
        "bqk": np.ascontiguousarray(bqk, dtype=f32),
        "bv": np.ascontiguousarray(bv, dtype=f32),
        "bproj": np.ascontiguousarray(proj_b, dtype=f32),
        "bfc1": np.ascontiguousarray(bfc1, dtype=f32),
        "bfc2": np.ascontiguousarray(fc2_b, dtype=f32),
    }


def _enable_axon_trace():
    """Register the NTFF profile hook that this image's antenv lacks."""
    import types
    from trn_agent_boot.trn_boot import _ntff_profile_via_ctypes
    mod = types.ModuleType("antenv.axon_hooks")
    hook = _ntff_profile_via_ctypes("/opt/axon/libaxon_pjrt.so")
    mod.get_axon_ntff_profile_hook = lambda: hook
    mod.set_axon_ntff_profile_hook = lambda h: None
    sys.modules["antenv.axon_hooks"] = mod
    import concourse.bass_utils as bu
    bu.upload_artifacts = lambda tmpdir: tmpdir  # no artifact bucket here


def _run_on_device(x_full, consts, trace=False):
    """Build + run the SPMD kernel on the 8 cores. Returns (out, exec_ns)."""
    sys.path.insert(0, "/opt/trn_rl_repo")
    from concourse.bass_utils import run_bass_kernel_spmd

    if os.environ.get("BASS_LDW_OPT", "0") == "1":
        import concourse.bass_utils as bu
        if not getattr(bu, "_ldw_patched", False):
            orig_run = bu.run_command

            def _run_ldw(argv, **kw):
                argv = ["--enable-ldw-opt=true"
                        if a == "--enable-ldw-opt=false" else a for a in argv]
                return orig_run(argv, **kw)

            bu.run_command = _run_ldw
            bu._ldw_patched = True

    tmpdir = None
    if trace:
        try:
            _enable_axon_trace()
            tmpdir = os.environ.get("BASS_KERNEL_TRACE_DIR")
            if tmpdir:
                import shutil
                shutil.rmtree(tmpdir, ignore_errors=True)
                os.makedirs(tmpdir, exist_ok=True)
        except Exception as e:  # profiling is best-effort
            print("trace hook setup failed:", e, file=sys.stderr)
            trace = False

    nc = _build_nc()
    in_maps = []
    for core in range(NCORES):
        m = dict(consts)
        m["xs"] = np.ascontiguousarray(
            x_full[core * PER:(core + 1) * PER], dtype=np.float32)
        in_maps.append(m)
    try:
        res = run_bass_kernel_spmd(nc, in_maps, list(range(NCORES)),
                                   trace=trace, tmpdir=tmpdir)
    except Exception:
        if not trace:
            raise
        print("traced run failed; retrying without trace", file=sys.stderr)
        res = run_bass_kernel_spmd(nc, in_maps, list(range(NCORES)), trace=False)
    out = np.concatenate([r["out"] for r in res.results], axis=0)
    return out, res.exec_time_ns


def _subproc_main(tmpdir):
    import ml_dtypes
    data = np.load(os.path.join(tmpdir, "in.npz"))
    consts = {}
    for k in data.files:
        if k == "x":
            continue
        if k.endswith("__bf16"):
            consts[k[:-6]] = data[k].view(ml_dtypes.bfloat16)
        else:
            consts[k] = data[k]
    trace = os.environ.get("BASS_KERNEL_TRACE", "0") == "1"
    out, exec_ns = _run_on_device(data["x"], consts, trace=trace)
    np.savez(os.path.join(tmpdir, "out.npz"), out=out,
             exec_ns=np.int64(exec_ns if exec_ns else -1))


LAST_EXEC_NS = None


def kernel(**inputs):
    global LAST_EXEC_NS
    x = np.asarray(inputs["x"], np.float32)
    assert x.shape == (B, N, C), x.shape
    t_h = int(np.asarray(inputs.get("t_h", 14)))
    t_w = int(np.asarray(inputs.get("t_w", 14)))
    assert t_h * t_w == TLEN, (t_h, t_w)
    consts = _prep_inputs(inputs)

    # Run the device part in a subprocess with a clean JAX platform env, so a
    # harness that pinned JAX_PLATFORMS=cpu (for the reference) doesn't break
    # the PJRT/axon execution path.
    import subprocess
    import tempfile
    with tempfile.TemporaryDirectory() as td:
        saved = {}
        for k, v in consts.items():
            if v.dtype == np.float32:
                saved[k] = v
            else:  # bfloat16 -> ship as uint16 bits
                saved[k + "__bf16"] = v.view(np.uint16)
        np.savez(os.path.join(td, "in.npz"), x=x, **saved)
        env = dict(os.environ)
        env.pop("JAX_PLATFORMS", None)
        pyp = env.get("PYTHONPATH", "")
        here = os.path.dirname(os.path.abspath(__file__))
        env["PYTHONPATH"] = ":".join(p for p in [here, "/opt/trn_rl_repo", pyp] if p)
        subprocess.run(
            [sys.executable, "-c",
             f"import kernel; kernel._subproc_main({td!r})"],
            check=True, env=env)
        data = np.load(os.path.join(td, "out.npz"))
        out = data["out"]
        LAST_EXEC_NS = int(data["exec_ns"])
    return out.astype(np.float32)


if __name__ == "__main__":
    if len(sys.argv) > 1 and sys.argv[1] == "_sub":
        _subproc_main(sys.argv[2])


# revision 38
# speedup vs baseline: 1.1868x; 1.0003x over previous
"""MixFormer block kernel for 8 Trainium2 NeuronCores.

Sharding: data-parallel over batch B=16 -> 2 batch elements per core.
No collectives needed. Each core runs the full block (LN1 -> mixed
attention -> proj residual -> LN2 -> MLP residual) on its 2 batch
elements with bf16 matmuls and fp32 accumulation/residual path.

kernel(**inputs) takes the FULL inputs (as produced by the reference
setup_inputs) and returns the FULL [16, 980, 768] fp32 output.
"""

import os
import sys
import numpy as np

# ---------------------------------------------------------------- constants
B, N, C = 16, 980, 768
H, HD, HID = 12, 64, 3072
TLEN = 196  # t_h * t_w template tokens; search tokens attend to all N
EPS = 1e-5
NCORES = 8
PER = B // NCORES  # batch elements per core

NT = (N + 127) // 128  # 8 token tiles (7x128 + 84)
TOK_TILES = [(i * 128, min(128, N - i * 128)) for i in range(NT)]
TOK_CHUNKS = [(0, 490), (490, 490)]             # moving-dim chunks over tokens
V_CHUNKS = [(0, 512), (512, 256)]               # chunks over C=768 outputs
TMPL_KT = [(0, 128), (128, TLEN - 128)]         # key tiles for template region
TMPL_QCH = [(0, TLEN)]                          # template query chunk
SRCH_QCH = [(TLEN, 392), (TLEN + 392, 392)]     # search query chunks (784 = 2x392)


def _build_nc(hoist=True):
    import concourse.bass as bass
    import concourse.tile as tile
    import concourse.mybir as mybir
    from concourse.masks import make_identity
    from contextlib import ExitStack

    f32 = mybir.dt.float32
    bf16 = mybir.dt.bfloat16
    AF = mybir.ActivationFunctionType
    OP = mybir.AluOpType

    nc = bass.Bass()

    xs = nc.dram_tensor("xs", [PER, N, C], f32, kind="ExternalInput")
    wqk = nc.dram_tensor("wqk", [C, 2 * C], bf16, kind="ExternalInput")
    wv = nc.dram_tensor("wv", [C, C], bf16, kind="ExternalInput")
    wproj = nc.dram_tensor("wproj", [C, C], bf16, kind="ExternalInput")
    wfc1 = nc.dram_tensor("wfc1", [C, HID], bf16, kind="ExternalInput")
    wfc2 = nc.dram_tensor("wfc2", [HID, C], bf16, kind="ExternalInput")
    bqk = nc.dram_tensor("bqk", [2 * C], f32, kind="ExternalInput")
    bv = nc.dram_tensor("bv", [C], f32, kind="ExternalInput")
    bproj = nc.dram_tensor("bproj", [C], f32, kind="ExternalInput")
    bfc1 = nc.dram_tensor("bfc1", [HID], f32, kind="ExternalInput")
    bfc2 = nc.dram_tensor("bfc2", [C], f32, kind="ExternalInput")
    out_d = nc.dram_tensor("out", [PER, N, C], f32, kind="ExternalOutput")
    xmid_d = nc.dram_tensor("xmid", [PER, N, C], f32)  # internal scratch
    dnb_d = nc.dram_tensor("dnb", [48, 2, 512], f32)   # denom bounce buffer
    dnb2_d = nc.dram_tensor("dnb2", [48, 2, 512], f32)  # reciprocal bounce

    def layernorm_to_T(tc, tp, pst, src, tsz, t0, dstT, ident, eps_sb):
        """src: [tsz, 768] fp32 SBUF AP -> dstT[:, :, t0:t0+tsz] feature-major bf16."""
        stats = tp.tile([128, 3, 6], f32, tag="ln_st")
        for g in range(3):
            nc.vector.bn_stats(out=stats[:tsz, g], in_=src[:, g * 256:(g + 1) * 256])
        mv = tp.tile([128, 2], f32, tag="ln_mv")
        nc.vector.bn_aggr(out=mv[:tsz], in_=stats[:tsz])
        # rstd = exp(-0.5 * ln(var + eps)); keeps ACT in the ln/exp table set
        lnv = tp.tile([128, 1], f32, tag="ln_lnv")
        nc.scalar.activation(out=lnv[:tsz], in_=mv[:tsz, 1:2], func=AF.Ln,
                             bias=eps_sb[:tsz])
        rstd = tp.tile([128, 1], f32, tag="ln_rstd")
        nc.scalar.activation(out=rstd[:tsz], in_=lnv[:tsz], func=AF.Exp, scale=-0.5)
        ctr = tp.tile([128, C], f32, tag="ln_ctr")
        nc.vector.tensor_scalar_sub(ctr[:tsz], src, mv[:tsz, 0:1])
        lnt = tp.tile([128, C], bf16, tag="ln_out")
        nc.vector.tensor_scalar_mul(lnt[:tsz], ctr[:tsz], rstd[:tsz, 0:1])
        for c in range(6):
            pt = pst.tile([128, 128], bf16, tag="ln_tr")
            nc.tensor.transpose(pt[:, :tsz], lnt[:tsz, c * 128:(c + 1) * 128],
                                ident[:tsz, :tsz])
            nc.scalar.copy(out=dstT[:, c, t0:t0 + tsz], in_=pt[:, :tsz])

    with tile.TileContext(nc) as tc, ExitStack() as top:
        persist = top.enter_context(tc.tile_pool(name="persist", bufs=1))
        ident = persist.tile([128, 128], bf16)
        make_identity(nc, ident)
        wqk_sb = persist.tile([128, 6, 2 * C], bf16)
        nc.sync.dma_start(out=wqk_sb, in_=wqk.rearrange("(k p) o -> p k o", p=128))
        wv_sb = persist.tile([128, 6, C], bf16)
        nc.sync.dma_start(out=wv_sb, in_=wv.rearrange("(k p) o -> p k o", p=128))
        wproj_sb = persist.tile([128, 6, C], bf16)
        nc.sync.dma_start(out=wproj_sb, in_=wproj.rearrange("(k p) o -> p k o", p=128))
        bqk_sb = persist.tile([128, 12], f32)
        nc.sync.dma_start(out=bqk_sb, in_=bqk.rearrange("(t p) -> p t", p=128))
        bfc1_sb = persist.tile([128, 24], f32)
        nc.sync.dma_start(out=bfc1_sb, in_=bfc1.rearrange("(t p) -> p t", p=128))
        bv_bc = persist.tile([128, C], f32)
        nc.sync.dma_start(out=bv_bc, in_=bv[:].partition_broadcast(128))
        bproj_bc = persist.tile([128, C], f32)
        nc.sync.dma_start(out=bproj_bc, in_=bproj[:].partition_broadcast(128))
        bfc2_bc = persist.tile([128, C], f32)
        nc.sync.dma_start(out=bfc2_bc, in_=bfc2[:].partition_broadcast(128))
        eps_sb = persist.tile([128, 1], f32)
        nc.vector.memset(eps_sb, EPS)

        for b in range(PER):
            with ExitStack() as bs:
                ln2p = bs.enter_context(tc.tile_pool(name=f"ln2p{b}", bufs=1))
                ln2T = ln2p.tile([128, 6, N], bf16)

                with ExitStack() as asx:
                    abuf = asx.enter_context(tc.tile_pool(name=f"abuf{b}", bufs=1))
                    x_sb = abuf.tile([128, NT, C], f32)
                    ln1T = abuf.tile([128, 6, N], bf16)
                    qkT = abuf.tile([128, 12, N], bf16)   # q o-tiles 0..5, k 6..11
                    v_sb = abuf.tile([128, NT, H, 65], bf16)  # col 64 = ones
                    xatt = abuf.tile([128, 6, N], bf16)   # feature-major attn out

                    # ---- A1: load x, LN1, transpose to feature-major ----
                    with ExitStack() as ph:
                        tp = ph.enter_context(tc.tile_pool(name=f"a1t{b}", bufs=6))
                        pst = ph.enter_context(
                            tc.tile_pool(name=f"a1p{b}", bufs=7, space="PSUM"))
                        for t, (t0, tsz) in enumerate(TOK_TILES):
                            nc.sync.dma_start(out=x_sb[:tsz, t], in_=xs[b, t0:t0 + tsz])
                            layernorm_to_T(tc, tp, pst, x_sb[:tsz, t], tsz, t0,
                                           ln1T, ident, eps_sb)

                    # ---- A2: qkv projections ----
                    with ExitStack() as ph:
                        psqk = ph.enter_context(
                            tc.tile_pool(name=f"a2q{b}", bufs=4, space="PSUM"))
                        psv = ph.enter_context(
                            tc.tile_pool(name=f"a2v{b}", bufs=4, space="PSUM"))
                        # q^T, k^T feature-major [o, tok]
                        for ot in range(12):
                            for (c0, csz) in TOK_CHUNKS:
                                ps = psqk.tile([128, 512], f32, tag="qk")
                                for k in range(6):
                                    nc.tensor.matmul(
                                        ps[:, :csz],
                                        wqk_sb[:, k, ot * 128:(ot + 1) * 128],
                                        ln1T[:, k, c0:c0 + csz],
                                        start=(k == 0), stop=(k == 5))
                                nc.vector.tensor_scalar_add(
                                    qkT[:, ot, c0:c0 + csz], ps[:, :csz],
                                    bqk_sb[:, ot:ot + 1])
                        # v token-major with per-head stride-65 layout + ones col
                        nc.vector.memset(v_sb[:, :, :, 64:65], 1.0)
                        for t, (t0, tsz) in enumerate(TOK_TILES):
                            for (c0, csz) in V_CHUNKS:
                                ps = psv.tile([128, 512], f32, tag="v")
                                for k in range(6):
                                    nc.tensor.matmul(
                                        ps[:tsz, :csz],
                                        ln1T[:, k, t0:t0 + tsz],
                                        wv_sb[:, k, c0:c0 + csz],
                                        start=(k == 0), stop=(k == 5))
                                nc.vector.tensor_add(
                                    v_sb[:tsz, t, c0 // 64:(c0 + csz) // 64, 0:64],
                                    ps[:tsz, :csz].rearrange("p (h d) -> p h d", d=64),
                                    bv_bc[:tsz, c0:c0 + csz].rearrange(
                                        "p (h d) -> p h d", d=64))

                    # ---- A3: attention per head pair ----
                    with ExitStack() as ph:
                        pss = ph.enter_context(
                            tc.tile_pool(name=f"a3s{b}", bufs=1, space="PSUM"))
                        psa = ph.enter_context(
                            tc.tile_pool(name=f"a3a{b}", bufs=1, space="PSUM"))
                        ptp = ph.enter_context(tc.tile_pool(name=f"a3p{b}", bufs=6))
                        dnp = ph.enter_context(tc.tile_pool(name=f"a3d{b}", bufs=3))
                        for hp in range(6):
                            for (kts, qchunks) in ((TMPL_KT, TMPL_QCH),
                                                   (TOK_TILES, SRCH_QCH)):
                                nch = len(qchunks)
                                av = [psa.tile([128, 2, 512], f32, tag=f"av{s}",
                                                name=f"av{s}")
                                      for s in (0, 1)]
                                nkt = len(kts)
                                for ki, (k0, ksz) in enumerate(kts):
                                    kt_t, ko = k0 // 128, k0 % 128
                                    qsz0 = qchunks[0][1]
                                    for s in (0, 1):
                                        pb = s * 64
                                        h = 2 * hp + s
                                        pt = ptp.tile([128, nch, 512], bf16,
                                                      tag=f"pt{s}")
                                        ss = pss.tile([128, 2, 512], f32,
                                                      tag=f"s{s}")
                                        for ci, (q0, qsz) in enumerate(qchunks):
                                            # S^T[kt,qt] = k_h^T.T @ q_h^T (K=64,
                                            # row group s) — pairs run concurrently
                                            nc.tensor.matmul(
                                                ss[:ksz, ci, :qsz],
                                                qkT[pb:pb + 64, 6 + hp, k0:k0 + ksz],
                                                qkT[pb:pb + 64, hp, q0:q0 + qsz],
                                                start=True, stop=True)
                                        # one exp over all chunks of this key tile
                                        nc.scalar.activation(
                                            pt[:ksz, :nch, :qsz0],
                                            ss[:ksz, :nch, :qsz0], AF.Exp)
                                        for ci, (q0, qsz) in enumerate(qchunks):
                                            # unnormalized AV; lhsT col 64 is ones
                                            # -> row 64 of psum = softmax denom
                                            nc.tensor.matmul(
                                                av[s][0:65, ci, :qsz],
                                                v_sb[ko:ko + ksz, kt_t, h, 0:65],
                                                pt[:ksz, ci, :qsz],
                                                start=(ki == 0), stop=(ki == nkt - 1))
                                # normalize: xatt_h = av[0:64] * (1/av[64])
                                for s in (0, 1):
                                    qsz0 = qchunks[0][1]
                                    # copy psum -> SBUF promptly so the next
                                    # head pair's AV matmuls get the banks
                                    avs = dnp.tile([65, 2, 512], f32,
                                                   tag=f"avs{s}")
                                    nc.vector.tensor_copy(
                                        avs[:, :nch, :qsz0],
                                        av[s][0:65, :nch, :qsz0])
                                    bc = dnp.tile([64, 2, 512], f32, tag="bc")
                                    di = ((b * 6 + hp) * 2 + (nch - 1)) * 2 + s
                                    # denom row -> DRAM, reload spread over 49
                                    # partitions so the 8-pass reciprocal runs
                                    # on a short free dim, then bounce back and
                                    # broadcast across 64 partitions (DMA-only)
                                    fr = qsz0 // 49
                                    nc.sync.dma_start(
                                        out=dnb_d[di:di + 1, :nch, :qsz0],
                                        in_=avs[64:65, :nch, :qsz0])
                                    dn_t = dnp.tile([49, 2, 16], f32, tag="dnt")
                                    nc.sync.dma_start(
                                        out=dn_t[:, :nch, :fr],
                                        in_=dnb_d[di, :nch, :qsz0].rearrange(
                                            "a (p f) -> p a f", p=49))
                                    dn_r = dnp.tile([49, 2, 16], f32, tag="dnr")
                                    nc.vector.reciprocal(dn_r[:, :nch, :fr],
                                                         dn_t[:, :nch, :fr])
                                    nc.sync.dma_start(
                                        out=dnb2_d[di, :nch, :qsz0].rearrange(
                                            "a (p f) -> p a f", p=49),
                                        in_=dn_r[:, :nch, :fr])
                                    nc.sync.dma_start(
                                        out=bc[:, :nch, :qsz0],
                                        in_=dnb2_d[di, :nch, :qsz0]
                                        .partition_broadcast(64))
                                    if s == 0:
                                        for ci, (q0, qsz) in enumerate(qchunks):
                                            nc.vector.tensor_mul(
                                                xatt[0:64, hp, q0:q0 + qsz],
                                                avs[0:64, ci, :qsz],
                                                bc[:, ci, :qsz])
                                    else:
                                        stg = dnp.tile([64, 2, 512], bf16, tag="stg")
                                        for ci, (q0, qsz) in enumerate(qchunks):
                                            nc.vector.tensor_mul(
                                                stg[:, ci, :qsz],
                                                avs[0:64, ci, :qsz],
                                                bc[:, ci, :qsz])
                                        for ci, (q0, qsz) in enumerate(qchunks):
                                            # odd head rows live at partitions 64+
                                            nc.sync.dma_start(
                                                out=xatt[64:128, hp, q0:q0 + qsz],
                                                in_=stg[:, ci, :qsz])

                    # ---- A4+A5: proj + residual -> xmid; LN2 ----
                    with ExitStack() as ph:
                        psp = ph.enter_context(
                            tc.tile_pool(name=f"a4p{b}", bufs=4, space="PSUM"))
                        pst = ph.enter_context(
                            tc.tile_pool(name=f"a4t{b}", bufs=4, space="PSUM"))
                        tp = ph.enter_context(tc.tile_pool(name=f"a4s{b}", bufs=6))
                        for t, (t0, tsz) in enumerate(TOK_TILES):
                            xm = tp.tile([128, C], f32, tag="xm")
                            for (c0, csz) in V_CHUNKS:
                                ps = psp.tile([128, 512], f32, tag="pj")
                                for k in range(6):
                                    nc.tensor.matmul(
                                        ps[:tsz, :csz],
                                        xatt[:, k, t0:t0 + tsz],
                                        wproj_sb[:, k, c0:c0 + csz],
                                        start=(k == 0), stop=(k == 5))
                                nc.vector.tensor_add(
                                    xm[:tsz, c0:c0 + csz], ps[:tsz, :csz],
                                    x_sb[:tsz, t, c0:c0 + csz])
                            nc.vector.tensor_add(xm[:tsz], xm[:tsz], bproj_bc[:tsz])
                            nc.sync.dma_start(out=xmid_d[b, t0:t0 + tsz], in_=xm[:tsz])
                            layernorm_to_T(tc, tp, pst, xm[:tsz], tsz, t0,
                                           ln2T, ident, eps_sb)

                # ---- B: MLP (attention buffers released) ----
                with ExitStack() as ph:
                    mw = ph.enter_context(tc.tile_pool(name=f"mw{b}", bufs=1))
                    wf1 = mw.tile([128, 6, HID], bf16)
                    nc.sync.dma_start(out=wf1,
                                      in_=wfc1.rearrange("(k p) o -> p k o", p=128))
                    wf2 = mw.tile([128, 24, C], bf16)
                    nc.sync.dma_start(out=wf2,
                                      in_=wfc2.rearrange("(k p) o -> p k o", p=128))
                    hT = mw.tile([128, 24, N], bf16)
                    ps1 = ph.enter_context(
                        tc.tile_pool(name=f"b1p{b}", bufs=2, space="PSUM"))
                    ps2 = ph.enter_context(
                        tc.tile_pool(name=f"b2p{b}", bufs=4, space="PSUM"))
                    tpm = ph.enter_context(tc.tile_pool(name=f"bt{b}", bufs=3))
                    for ot in range(24):
                        ps = ps1.tile([128, 2, 512], f32, tag="f1")
                        for ci, (c0, csz) in enumerate(TOK_CHUNKS):
                            for k in range(6):
                                nc.tensor.matmul(
                                    ps[:, ci, :csz],
                                    wf1[:, k, ot * 128:(ot + 1) * 128],
                                    ln2T[:, k, c0:c0 + csz],
                                    start=(k == 0), stop=(k == 5))
                        nc.scalar.activation(
                            hT[:, ot, :].rearrange("p (a c) -> p a c", c=490),
                            ps[:, :, :490], AF.Gelu,
                            bias=bfc1_sb[:, ot:ot + 1])
                    for t, (t0, tsz) in enumerate(TOK_TILES):
                        xm = tpm.tile([128, C], f32, tag="xm2")
                        nc.sync.dma_start(out=xm[:tsz], in_=xmid_d[b, t0:t0 + tsz])
                        ot_t = tpm.tile([128, C], f32, tag="ott")
                        for (c0, csz) in V_CHUNKS:
                            ps = ps2.tile([128, 512], f32, tag="f2")
                            for k in range(24):
                                nc.tensor.matmul(
                                    ps[:tsz, :csz],
                                    hT[:, k, t0:t0 + tsz],
                                    wf2[:, k, c0:c0 + csz],
                                    start=(k == 0), stop=(k == 23))
                            nc.vector.tensor_add(
                                ot_t[:tsz, c0:c0 + csz], ps[:tsz, :csz],
                                xm[:tsz, c0:c0 + csz])
                        nc.vector.tensor_add(ot_t[:tsz], ot_t[:tsz], bfc2_bc[:tsz])
                        nc.sync.dma_start(out=out_d[b, t0:t0 + tsz], in_=ot_t[:tsz])

    if hoist:
        _hoist_excess_waits(nc, mybir)
    return nc


def _hoist_excess_waits(nc, mybir, cap=1, nop_cap=1):
    """walrus's 64B instruction encodings fit only ~1 sync-wait command for
    operand-heavy structs (TS/AC/...). Move excess waits onto same-engine
    NoOps inserted right before the instruction."""
    skip = ("InstNoOp", "InstEventSemaphore", "InstCall",
            "InstAllEngineBarrier", "InstUnconditionalBranch", "InstISA")
    n = 0
    for f in nc.m.functions:
        for blk in getattr(f, "blocks", []):
            out = []
            for inst in blk.instructions:
                si = inst.sync_info
                if (si is not None and len(si.on_wait) > cap
                        and type(inst).__name__ not in skip):
                    waits = list(si.on_wait)
                    keep, extra = waits[:cap], waits[cap:]
                    while extra:
                        chunk, extra = extra[:nop_cap], extra[nop_cap:]
                        n += 1
                        out.append(mybir.InstNoOp(
                            name=f"nopw-{n}", engine=inst.engine, ins=[], outs=[],
                            sync_info=mybir.SyncInfo(on_wait=chunk, on_update=[])))
                    inst.sync_info = mybir.SyncInfo(
                        on_wait=keep, on_update=list(si.on_update))
                out.append(inst)
            blk.instructions = out


def _prep_inputs(inputs):
    """Host-side weight folding; returns dict of per-core-constant arrays."""
    import ml_dtypes
    f32 = np.float32
    ln1_w = np.asarray(inputs["ln1_w"], f32)
    ln1_b = np.asarray(inputs["ln1_b"], f32)
    ln2_w = np.asarray(inputs["ln2_w"], f32)
    ln2_b = np.asarray(inputs["ln2_b"], f32)
    qkv_w = np.asarray(inputs["qkv_w"], f32)
    qkv_b = np.asarray(inputs["qkv_b"], f32)
    proj_w = np.asarray(inputs["proj_w"], f32)
    proj_b = np.asarray(inputs["proj_b"], f32)
    fc1_w = np.asarray(inputs["fc1_w"], f32)
    fc1_b = np.asarray(inputs["fc1_b"], f32)
    fc2_w = np.asarray(inputs["fc2_w"], f32)
    fc2_b = np.asarray(inputs["fc2_b"], f32)

    scale = HD ** -0.5
    # fold LN1 affine into qkv; fold attention scale into q
    w_full = ln1_w[:, None] * qkv_w            # [C, 3C]
    b_full = qkv_b + ln1_b @ qkv_w             # [3C]
    w_full = w_full.copy()
    b_full = b_full.copy()
    w_full[:, :C] *= scale
    b_full[:C] *= scale
    wqk = w_full[:, :2 * C]
    bqk = b_full[:2 * C]
    wv = w_full[:, 2 * C:]
    bv = b_full[2 * C:]
    # fold LN2 affine into fc1
    wfc1 = ln2_w[:, None] * fc1_w
    bfc1 = fc1_b + ln2_b @ fc1_w

    bf16 = ml_dtypes.bfloat16
    return {
        "wqk": np.ascontiguousarray(wqk, dtype=bf16),
        "wv": np.ascontiguousarray(wv, dtype=bf16),
        "wproj": np.ascontiguousarray(proj_w, dtype=bf16),
        "wfc1": np.ascontiguousarray(wfc1, dtype=bf16),
        "wfc2": np.ascontiguousarray(fc2_w, dtype=bf16),
        "bqk": np.ascontiguousarray(bqk, dtype=f32),
        "bv": np.ascontiguousarray(bv, dtype=f32),
        "bproj": np.ascontiguousarray(proj_b, dtype=f32),
        "bfc1": np.ascontiguousarray(bfc1, dtype=f32),
        "bfc2": np.ascontiguousarray(fc2_b, dtype=f32),
    }


def _enable_axon_trace():
    """Register the NTFF profile hook that this image's antenv lacks."""
    import types
    from trn_agent_boot.trn_boot import _ntff_profile_via_ctypes
    mod = types.ModuleType("antenv.axon_hooks")
    hook = _ntff_profile_via_ctypes("/opt/axon/libaxon_pjrt.so")
    mod.get_axon_ntff_profile_hook = lambda: hook
    mod.set_axon_ntff_profile_hook = lambda h: None
    sys.modules["antenv.axon_hooks"] = mod
    import concourse.bass_utils as bu
    bu.upload_artifacts = lambda tmpdir: tmpdir  # no artifact bucket here


def _run_on_device(x_full, consts, trace=False):
    """Build + run the SPMD kernel on the 8 cores. Returns (out, exec_ns)."""
    sys.path.insert(0, "/opt/trn_rl_repo")
    from concourse.bass_utils import run_bass_kernel_spmd

    if os.environ.get("BASS_LDW_OPT", "0") == "1":
        import concourse.bass_utils as bu
        if not getattr(bu, "_ldw_patched", False):
            orig_run = bu.run_command

            def _run_ldw(argv, **kw):
                argv = ["--enable-ldw-opt=true"
                        if a == "--enable-ldw-opt=false" else a for a in argv]
                return orig_run(argv, **kw)

            bu.run_command = _run_ldw
            bu._ldw_patched = True

    tmpdir = None
    if trace:
        try:
            _enable_axon_trace()
            tmpdir = os.environ.get("BASS_KERNEL_TRACE_DIR")
            if tmpdir:
                import shutil
                shutil.rmtree(tmpdir, ignore_errors=True)
                os.makedirs(tmpdir, exist_ok=True)
        except Exception as e:  # profiling is best-effort
            print("trace hook setup failed:", e, file=sys.stderr)
            trace = False

    nc = _build_nc()
    in_maps = []
    for core in range(NCORES):
        m = dict(consts)
        m["xs"] = np.ascontiguousarray(
            x_full[core * PER:(core + 1) * PER], dtype=np.float32)
        in_maps.append(m)
    try:
        res = run_bass_kernel_spmd(nc, in_maps, list(range(NCORES)),
                                   trace=trace, tmpdir=tmpdir)
    except Exception:
        if not trace:
            raise
        print("traced run failed; retrying without trace", file=sys.stderr)
        res = run_bass_kernel_spmd(nc, in_maps, list(range(NCORES)), trace=False)
    out = np.concatenate([r["out"] for r in res.results], axis=0)
    return out, res.exec_time_ns


def _subproc_main(tmpdir):
    import ml_dtypes
    data = np.load(os.path.join(tmpdir, "in.npz"))
    consts = {}
    for k in data.files:
        if k == "x":
            continue
        if k.endswith("__bf16"):
            consts[k[:-6]] = data[k].view(ml_dtypes.bfloat16)
        else:
            consts[k] = data[k]
    trace = os.environ.get("BASS_KERNEL_TRACE", "0") == "1"
    out, exec_ns = _run_on_device(data["x"], consts, trace=trace)
    np.savez(os.path.join(tmpdir, "out.npz"), out=out,
             exec_ns=np.int64(exec_ns if exec_ns else -1))


LAST_EXEC_NS = None


def kernel(**inputs):
    global LAST_EXEC_NS
    x = np.asarray(inputs["x"], np.float32)
    assert x.shape == (B, N, C), x.shape
    t_h = int(np.asarray(inputs.get("t_h", 14)))
    t_w = int(np.asarray(inputs.get("t_w", 14)))
    assert t_h * t_w == TLEN, (t_h, t_w)
    consts = _prep_inputs(inputs)

    # Run the device part in a subprocess with a clean JAX platform env, so a
    # harness that pinned JAX_PLATFORMS=cpu (for the reference) doesn't break
    # the PJRT/axon execution path.
    import subprocess
    import tempfile
    with tempfile.TemporaryDirectory() as td:
        saved = {}
        for k, v in consts.items():
            if v.dtype == np.float32:
                saved[k] = v
            else:  # bfloat16 -> ship as uint16 bits
                saved[k + "__bf16"] = v.view(np.uint16)
        np.savez(os.path.join(td, "in.npz"), x=x, **saved)
        env = dict(os.environ)
        env.pop("JAX_PLATFORMS", None)
        pyp = env.get("PYTHONPATH", "")
        here = os.path.dirname(os.path.abspath(__file__))
        env["PYTHONPATH"] = ":".join(p for p in [here, "/opt/trn_rl_repo", pyp] if p)
        subprocess.run(
            [sys.executable, "-c",
             f"import kernel; kernel._subproc_main({td!r})"],
            check=True, env=env)
        data = np.load(os.path.join(td, "out.npz"))
        out = data["out"]
        LAST_EXEC_NS = int(data["exec_ns"])
    return out.astype(np.float32)


if __name__ == "__main__":
    if len(sys.argv) > 1 and sys.argv[1] == "_sub":
        _subproc_main(sys.argv[2])
